# revision 24
# baseline (speedup 1.0000x reference)
"""BayesianAttention (power-law prior + causal mask) on 8 trn2 cores.

Self-contained: builds a Bass/Tile kernel, shards heads across 8 NeuronCores
(2 heads per core; wq/wk/wv column-sharded, wo row-sharded), runs via a
jitted 8-core PJRT runner, and reduces the partial outputs on host.

Device-side layout is fully transposed (contraction dims on partitions) and
all matmul operands are bf16 (f32 PSUM accumulation):
  host sends x^T [c, i] bf16; device computes q^T/k^T/v^T = W^T x^T,
  transposes v, s^T[j,i] = k^T_j . q^T_i (raw, no bias),
  probs = exp(s^T) * EG, where EG = exp(prior + causal mask) is a
  precomputed per-head Toeplitz table ([128, 4096] bf16, masked entries
  exactly 0), o^T[d,i] = v^T probs / (w^T probs), out^T[e,i] = wo^T o^T.
Host returns sum_c(out^T_c)^T.

The kernel is emitted as one software-pipelined loop over 512-wide query
blocks: projections(ib) -> attention(ib, both heads) -> output(ib), which
keeps the tensor engine continuously busy (PSUM pool backpressure paces the
phases); x / out move as a few large batched DMAs on the two HWDGE queues.
"""

import math
import os

import ml_dtypes
import numpy as np

S = 2048          # sequence length
DIM = 2048        # model dim
H = 16            # heads
HD = 128          # head dim
N_CORES = 8
HL = H // N_CORES  # heads per core (2)
DL = HL * HD       # local projected dim (256)
IB = 512           # i-block (query block, moving free dim)
NIB = S // IB
NJT = S // 128     # key tiles of 128
GW = 4096          # EG table width (needs >= S + IB)
EPS = 1e-5
MASKED_THRESH = -1e8   # additive mask values below this mean "fully masked"

TRACE = bool(int(os.environ.get("KBA_TRACE", "0")))

LAG = 6  # scores->o-matmul emission lag (bounds live probs tiles)

LAST_RUN_INFO = {}

MM_DTYPE = "bf16"  # kept for test.py printout


# ---------------------------------------------------------------- tile patch
def _apply_tile_patch():
    """walrus CoreV3 codegen tolerates only one sync-wait on an InstDrain;
    the tile-exit drain waits on the whole global clock. Spread the waits
    across extra SP nops."""
    import concourse.tile as tile
    from concourse import mybir
    from concourse.vector_clock import ScopedClock

    if getattr(tile.TileContext, "_kba_patched", False):
        return

    def _drain_and_barrier(self, tick_clock, wait_clock):
        nc = self.nc
        drain_inst = nc.sync.drain()
        wait_clock.add_sem_waits(
            drain_inst.ins, ScopedClock({None: tick_clock.global_clock})
        )
        si = drain_inst.ins.sync_info
        waits = list(si.on_wait or [])
        if len(waits) > 1:
            si.on_wait = waits[:1]
            for i in range(1, len(waits)):
                nop = nc.sync.nop(nofuse=True)
                nop.ins.sync_info = mybir.SyncInfo(
                    on_wait=waits[i : i + 1], on_update=[]
                )
        nc.all_engine_barrier()
        assert self.sems is not None
        popped = nc._tile_sem_poison_stack.pop()
        assert popped is self._sem_poison
        nc.clear_and_free_semaphores(list(self.sems.allocated().values()))
        nc.all_engine_barrier()

    tile.TileContext._drain_and_barrier = _drain_and_barrier
    tile.TileContext._kba_patched = True

    try:
        import concourse.tile_utils as tile_utils

        tile_utils.max_sbuf_usage = 208 * 1024
    except Exception:
        pass


# ------------------------------------------------------------- host helpers
def _toeplitz_profile(m2):
    """If mask[i, j] == phi(j - i) for all i,j, return phi (length 2S-1,
    index t + S - 1), else None."""
    phi = np.empty(2 * S - 1, dtype=np.float32)
    phi[S - 1 :] = m2[0, :]
    phi[: S - 1] = m2[1:, 0][::-1]
    idx = (np.arange(S)[None, :] - np.arange(S)[:, None]) + (S - 1)
    if np.array_equal(phi[idx], m2):
        return phi
    return None


def _eg_table(head, shape, scale, loc, start_pos, phi):
    """[128, GW] bf16: EG[p, u] = exp(prior(d) + phi(-d)), where
    d = i - j = u - p - (S - 1). Masked / out-of-range entries are 0."""
    p = np.arange(128, dtype=np.int64)[:, None]
    u = np.arange(GW, dtype=np.int64)[None, :]
    d = u - p - (S - 1)          # i - j
    dist = (-d - start_pos).astype(np.float32)  # k_pos - q_pos
    sh = np.float32(shape[0, head, 0, 0])
    sc = np.float32(scale[0, head, 0, 0])
    lo = np.float32(loc[0, head, 0, 0])
    loc_t = np.float32(np.exp(lo) - np.exp(-lo))
    z = (dist - loc_t) * np.exp(sc, dtype=np.float32)
    g = -np.power(np.abs(z) + np.float32(EPS), sh, dtype=np.float32)
    if phi is not None:
        t = np.clip(-d + (S - 1), 0, 2 * S - 2)
        g = g + phi[t]
        g[(-d < -(S - 1)) | (-d > (S - 1))] = -np.inf  # out of range: exp -> 0
    with np.errstate(over="ignore", under="ignore"):
        eg = np.exp(g, dtype=np.float32)
    return np.ascontiguousarray(eg.astype(ml_dtypes.bfloat16))


def _kept_tiles(m2):
    """kept[ib] = list of key-tile indices jt whose [128 x IB] block is not
    fully masked. Must be identical for every core (single SPMD program)."""
    kept = []
    for ib in range(NIB):
        row = []
        for jt in range(NJT):
            blk = m2[ib * IB : (ib + 1) * IB, jt * 128 : (jt + 1) * 128]
            if blk.max() > MASKED_THRESH:
                row.append(jt)
        kept.append(row)
    return kept


BAND_THR = 30.0  # drop key tiles once the prior bias is below -BAND_THR


def _prior_reach(head, shape, scale, loc):
    """Smallest distance d* such that the prior penalty is <= -BAND_THR for
    every d >= d* (attention weight < e^-30 of an undecayed key). 2*S if the
    prior never decays that far."""
    sh = np.float32(shape[0, head, 0, 0])
    sc = np.float32(scale[0, head, 0, 0])
    lo = np.float32(loc[0, head, 0, 0])
    loc_t = np.exp(lo) - np.exp(-lo)
    d = np.arange(2 * S, dtype=np.float32)
    pen = (np.abs(-d - loc_t) * np.exp(sc) + np.float32(EPS)) ** sh
    ok = pen >= BAND_THR
    if not ok.any():
        return 2 * S
    # first index from which ok holds for all larger d
    rev_ok = np.minimum.accumulate(ok[::-1])[::-1]
    idx = np.argmax(rev_ok)
    if not rev_ok[idx]:
        return 2 * S
    return int(idx)


HEADS_OF_CORE = [[c, c + H // 2] for c in range(N_CORES)]
# Slot s of every core runs the same instruction stream; slot 0 holds the
# steep-slope (short-reach) heads 0..7 so its key-tile band can be cut
# uniformly, slot 1 holds the flat heads 8..15 (full causal reach).


def _banded_kept(kept, shape, scale, loc):
    """Per-slot kept-tile lists: drop tiles whose whole [IB x 128] block is
    beyond every covered head's prior reach. Identical across cores by
    construction (slot reach = max over cores)."""
    kept_slots = []
    for s in range(HL):
        reach = max(
            _prior_reach(HEADS_OF_CORE[c][s], shape, scale, loc)
            for c in range(N_CORES)
        )
        rows = []
        for ib in range(NIB):
            i0 = ib * IB
            rows.append(
                tuple(
                    jt for jt in kept[ib] if jt * 128 + 127 >= i0 - reach
                )
            )
        kept_slots.append(tuple(rows))
    return tuple(kept_slots)


# ------------------------------------------------------------ program build
_PROGRAM_CACHE = {}


def _build_program(kept_key, fold_mask, repeat=1):
    key = (kept_key, fold_mask, repeat)
    if key in _PROGRAM_CACHE:
        return _PROGRAM_CACHE[key]

    import concourse.bass as bass
    import concourse.tile as tile
    from concourse import bacc, mybir
    from concourse.masks import make_identity

    _apply_tile_patch()

    f32 = mybir.dt.float32
    bf16 = mybir.dt.bfloat16

    kept = [[list(row) for row in slot_rows] for slot_rows in kept_key]

    nc = bacc.Bacc(
        "TRN2", target_bir_lowering=False, debug=False, num_devices=N_CORES
    )
    xT_d = nc.dram_tensor("xT", [S, S], bf16, kind="ExternalInput")
    wq_d = nc.dram_tensor("wq", [S, DL], bf16, kind="ExternalInput")
    wk_d = nc.dram_tensor("wk", [S, DL], bf16, kind="ExternalInput")
    wv_d = nc.dram_tensor("wv", [S, DL], bf16, kind="ExternalInput")
    wo_d = nc.dram_tensor("wo", [DL, S], bf16, kind="ExternalInput")
    g_d = nc.dram_tensor("g", [HL, 128, GW], bf16, kind="ExternalInput")
    if not fold_mask:
        maskT_d = nc.dram_tensor("maskT", [S, S], f32, kind="ExternalInput")
    outT_d = nc.dram_tensor("outT", [S, S], bf16, kind="ExternalOutput")

    Exp = mybir.ActivationFunctionType.Exp
    Copy = mybir.ActivationFunctionType.Copy

    with tile.TileContext(nc) as tc:
        import contextlib

        with contextlib.ExitStack() as ctx:
            consts = ctx.enter_context(tc.tile_pool(name="consts", bufs=1))
            persist = ctx.enter_context(tc.tile_pool(name="persist", bufs=1))
            xpool = ctx.enter_context(tc.tile_pool(name="xp", bufs=2))
            qpool = ctx.enter_context(tc.tile_pool(name="qp", bufs=2))
            vtpool = ctx.enter_context(tc.tile_pool(name="vt", bufs=2))
            pbpool = ctx.enter_context(tc.tile_pool(name="pb", bufs=3))
            prpool = ctx.enter_context(tc.tile_pool(name="pr", bufs=LAG + 2))
            rpool = ctx.enter_context(tc.tile_pool(name="rp", bufs=2))
            ospool = ctx.enter_context(tc.tile_pool(name="os", bufs=2))
            obpool = ctx.enter_context(tc.tile_pool(name="ob", bufs=2))
            mpool = (
                ctx.enter_context(tc.tile_pool(name="mk", bufs=3))
                if not fold_mask
                else None
            )
            # PSUM: "big" (6 banks) round-robins phase-1 projection
            # accumulators (4+2), phase-2 scores tiles, and phase-3 output
            # tiles. "acc" (2 banks) serves v-transposes (phase 1) and the
            # o/softmax-sum accumulators (phase 2).
            big = ctx.enter_context(tc.tile_pool(name="big", bufs=6, space="PSUM"))
            acc = ctx.enter_context(tc.tile_pool(name="acc", bufs=2, space="PSUM"))

            # ---- constants / weights (batched DMAs, bf16 direct) ----
            # Split w into chunk-halves ordered by first use so the first
            # projection matmuls start ~4us earlier.
            wq_sb = consts.tile([128, NJT, DL], bf16, name="wq")
            wk_sb = consts.tile([128, NJT, DL], bf16, name="wk")
            wv_sb = consts.tile([128, NJT, DL], bf16, name="wv")
            wo_sb = consts.tile([128, HL, S], bf16, name="wo")
            g_sb = consts.tile([128, HL, GW], bf16, name="g")
            hc = NJT // 2
            for w_sb, w_d in ((wq_sb, wq_d), (wk_sb, wk_d)):
                nc.scalar.dma_start(
                    out=w_sb[:, :hc, :],
                    in_=w_d.ap()[: hc * 128, :].rearrange("(c p) d -> p c d", p=128),
                )
            for w_sb, w_d in ((wq_sb, wq_d), (wk_sb, wk_d)):
                nc.scalar.dma_start(
                    out=w_sb[:, hc:, :],
                    in_=w_d.ap()[hc * 128 :, :].rearrange("(c p) d -> p c d", p=128),
                )
            nc.scalar.dma_start(
                out=wv_sb[:], in_=wv_d.ap().rearrange("(c p) d -> p c d", p=128)
            )
            nc.scalar.dma_start(
                out=wo_sb[:], in_=wo_d.ap().rearrange("(h p) e -> p h e", p=128)
            )
            nc.scalar.dma_start(
                out=g_sb[:], in_=g_d.ap().rearrange("h p u -> p h u")
            )
            # [128, 128] all-ones stationary: the softmax-sum matmul then
            # produces Sum broadcast across all 128 partitions at no extra
            # PE cost (cycles scale with the moving width, not stationary m).
            ones_sb = consts.tile([128, 128], bf16, name="ones_sb")
            nc.vector.memset(ones_sb[:], 1.0)
            ident = consts.tile([128, 128], bf16)
            make_identity(nc, ident[:])

            kT = persist.tile([128, HL, S], bf16)          # [d, h, j]
            v_sb = persist.tile([128, HL, NJT, HD], bf16)  # [j, h, jt, d]

            qTb_of = {}
            o_sb_of = {}
            xt_of = {}

            def emit_xt_dma(ib):
                """Prefetch x^T for i-block ib (issued a pipeline step ahead
                of its pass-A consumers so the transfer is fully hidden)."""
                isl = bass.ts(ib, IB)
                xt = xpool.tile([128, NJT, IB], bf16, name="xt")
                xt_of[ib] = xt
                half = NJT // 2
                nc.sync.dma_start(
                    out=xt[:, :half, :],
                    in_=xT_d.ap()[: half * 128, isl].rearrange(
                        "(c p) i -> p c i", p=128
                    ),
                )
                nc.sync.dma_start(
                    out=xt[:, half:, :],
                    in_=xT_d.ap()[half * 128 :, isl].rearrange(
                        "(c p) i -> p c i", p=128
                    ),
                )

            def phase1_passA_units(ib):
                """Generator of pass-A work units for i-block ib: q^T/k^T
                projection matmuls per contraction chunk, then the qTb/kT
                copies. Yield granularity ~1.3us of PE work so units can be
                interleaved into phase 2 as PE filler."""
                isl = bass.ts(ib, IB)
                xt = xt_of[ib]
                qk_ps = {}
                for proj in range(2):
                    for dt_i in range(HL):
                        qk_ps[(proj, dt_i)] = big.tile(
                            [128, IB], f32, tag="big", name=f"ps{proj}{dt_i}"
                        )
                for ct in range(NJT):
                    for proj, w_sb in enumerate((wq_sb, wk_sb)):
                        for dt_i in range(HL):
                            nc.tensor.matmul(
                                qk_ps[(proj, dt_i)][:],
                                lhsT=w_sb[:, ct, dt_i * HD : (dt_i + 1) * HD],
                                rhs=xt[:, ct, :],
                                start=(ct == 0),
                                stop=(ct == NJT - 1),
                            )
                    yield
                qTb = qpool.tile([128, HL, IB], bf16, name="qTb")
                qTb_of[ib] = qTb
                for dt_i in range(HL):
                    nc.scalar.activation(qTb[:, dt_i, :], qk_ps[(0, dt_i)][:], Copy)
                    nc.vector.tensor_copy(kT[:, dt_i, isl], qk_ps[(1, dt_i)][:])
                yield

            def emit_phase1_passB(ib):
                """v^T projection + v transpose into v_sb for i-block ib.
                Emitted between phase2(prev) and phase3(prev) to fill the
                softmax-tail join with independent PE work."""
                xt = xt_of.pop(ib)
                v_ps = [
                    big.tile([128, IB], f32, tag="big", name=f"vps{dt_i}")
                    for dt_i in range(HL)
                ]
                for ct in range(NJT):
                    for dt_i in range(HL):
                        nc.tensor.matmul(
                            v_ps[dt_i][:],
                            lhsT=wv_sb[:, ct, dt_i * HD : (dt_i + 1) * HD],
                            rhs=xt[:, ct, :],
                            start=(ct == 0),
                            stop=(ct == NJT - 1),
                        )
                for dt_i in range(HL):
                    vt = vtpool.tile([128, IB], bf16, name="vt")
                    nc.scalar.activation(vt[:], v_ps[dt_i][:], Copy)
                    # transpose v^T [d, j] -> v [j, d] in 128-blocks
                    for s4 in range(IB // 128):
                        jt = (ib * IB) // 128 + s4
                        tp = acc.tile([128, 128], bf16, tag="acc", name="tp")
                        nc.tensor.transpose(
                            tp[:], vt[:, s4 * 128 : (s4 + 1) * 128], ident[:]
                        )
                        nc.vector.tensor_copy(v_sb[:, dt_i, jt, :], tp[:])

            def emit_phase2(ib, filler=None):
                """Attention for both local heads of i-block ib. `filler` is
                an optional iterator of next-block pass-A units, drained
                evenly across the score tiles to give PE independent work
                while the exp/multiply chain catches up."""
                isl = bass.ts(ib, IB)
                qTb = qTb_of[ib]
                o_sb = ospool.tile([128, HL, IB], bf16, name="o_sb")
                o_sb_of[ib] = o_sb
                n_tiles = sum(len(kept[h][ib]) for h in range(HL))
                tiles_done = 0
                filler_done = 0
                filler_total = NJT + 1  # passA yields per ct chunk + copies

                def drain_filler():
                    nonlocal filler_done
                    if filler is None:
                        return
                    while filler_done / filler_total <= tiles_done / n_tiles:
                        try:
                            next(filler)
                            filler_done += 1
                        except StopIteration:
                            break
                for h in range(HL):
                    jts = kept[h][ib]
                    # Causal narrowing: a diagonal key tile jt only affects
                    # queries i >= jt*128, i.e. moving columns [off:], with
                    # off = jt*128 - ib*IB. Skipped columns stay correct: the
                    # first tile (off 0) writes the accumulators full-width,
                    # and no later tile contributes below its own off.
                    offs = [max(0, jt * 128 - ib * IB) for jt in jts]
                    if offs[0] != 0:  # first tile must init accums fully
                        offs = [0] * len(jts)
                    oacc = acc.tile([128, IB], f32, tag="acc", name="oacc")
                    sacc = acc.tile([128, IB], f32, tag="acc", name="sacc")
                    probs = {}

                    def emit_pv(idx):
                        jt = jts[idx]
                        off = offs[idx]
                        nc.tensor.matmul(
                            oacc[:, off:],
                            lhsT=v_sb[:, h, jt, :],
                            rhs=probs[idx][:, off:],
                            start=(idx == 0),
                            stop=(idx == len(jts) - 1),
                        )
                        nc.tensor.matmul(
                            sacc[:, off:],
                            lhsT=ones_sb[:],
                            rhs=probs[idx][:, off:],
                            start=(idx == 0),
                            stop=(idx == len(jts) - 1),
                        )

                    for idx, jt in enumerate(jts):
                        drain_filler()
                        tiles_done += 1
                        off = offs[idx]
                        sc = big.tile([128, IB], f32, tag="big", name="sc")
                        nc.tensor.matmul(
                            sc[:, off:],
                            lhsT=kT[:, h, jt * 128 : (jt + 1) * 128],
                            rhs=qTb[:, h, off:],
                            start=True,
                            stop=True,
                        )
                        if not fold_mask:
                            mt = mpool.tile([128, IB], f32)
                            nc.sync.dma_start(
                                out=mt[:],
                                in_=maskT_d[jt * 128 : (jt + 1) * 128, isl],
                            )
                            nc.vector.tensor_add(
                                sc[:, off:], sc[:, off:], mt[:, off:]
                            )
                        pb = pbpool.tile([128, IB], bf16, name="pb")
                        nc.scalar.activation(pb[:, off:], sc[:, off:], Exp)
                        base = ib * IB - jt * 128 + (S - 1)
                        pr = prpool.tile([128, IB], bf16, name="pr")
                        nc.vector.tensor_mul(
                            pr[:, off:],
                            pb[:, off:],
                            g_sb[:, h, base + off : base + IB],
                        )
                        probs[idx] = pr
                        if idx - LAG >= 0:
                            emit_pv(idx - LAG)
                    for idx in range(max(0, len(jts) - LAG), len(jts)):
                        emit_pv(idx)

                    rbc = rpool.tile([128, IB], f32, tag="rbc", name="rbc")
                    nc.vector.reciprocal(rbc[:], sacc[:])
                    nc.vector.tensor_mul(o_sb[:, h, :], oacc[:], rbc[:])
                if filler is not None:
                    for _ in filler:
                        pass

            def emit_phase3(ib):
                """out^T = wo^T o^T for i-block ib (partial; host sums cores).
                Output DMA goes out in two halves so the transfer overlaps the
                remaining copies."""
                isl = bass.ts(ib, IB)
                o_sb = o_sb_of.pop(ib)
                qTb_of.pop(ib, None)
                ob = obpool.tile([128, NJT, IB], bf16, name="ob")
                halves = (range(0, NJT // 2), range(NJT // 2, NJT))
                for hi, ets in enumerate(halves):
                    for et in ets:
                        po = big.tile([128, IB], f32, tag="big", name="po")
                        for h in range(HL):
                            nc.tensor.matmul(
                                po[:],
                                lhsT=wo_sb[:, h, et * 128 : (et + 1) * 128],
                                rhs=o_sb[:, h, :],
                                start=(h == 0),
                                stop=(h == HL - 1),
                            )
                        if et % 2 == 0:
                            nc.scalar.activation(ob[:, et, :], po[:], Copy)
                        else:
                            nc.vector.tensor_copy(ob[:, et, :], po[:])
                    e0, e1 = ets[0], ets[-1] + 1
                    nc.sync.dma_start(
                        out=outT_d.ap()[e0 * 128 : e1 * 128, isl].rearrange(
                            "(e p) i -> p e i", p=128
                        ),
                        in_=ob[:, e0:e1, :],
                    )

            # Software-pipelined schedule: the next block's pass-A projection
            # units are interleaved INTO attention(ib) as PE filler, its
            # v-projection (pass B) fills the softmax-tail -> output join,
            # and x prefetches run a full step ahead.
            blocks = [ib for _rep in range(repeat) for ib in range(NIB)]
            emit_xt_dma(blocks[0])
            for _ in phase1_passA_units(blocks[0]):
                pass
            if len(blocks) > 1:
                emit_xt_dma(blocks[1])
            emit_phase1_passB(blocks[0])
            for n, ib in enumerate(blocks):
                nxt = blocks[n + 1] if n + 1 < len(blocks) else None
                emit_phase2(
                    ib,
                    filler=phase1_passA_units(nxt) if nxt is not None else None,
                )
                if nxt is not None:
                    emit_phase1_passB(nxt)
                if n + 2 < len(blocks):
                    emit_xt_dma(blocks[n + 2])
                emit_phase3(ib)

    nc.compile()
    _PROGRAM_CACHE[key] = nc
    return nc


# ------------------------------------------------------------------- kernel
def prepare(x, mask, wq, wk, wv, wo, shape, scale, loc, start_pos):
    """Host prep: build/cache program and per-core input maps."""
    x32 = np.asarray(x, np.float32).reshape(S, DIM)
    m2 = np.asarray(mask, np.float32).reshape(S, S)
    wq32 = np.asarray(wq, np.float32)
    wk32 = np.asarray(wk, np.float32)
    wv32 = np.asarray(wv, np.float32)
    wo32 = np.asarray(wo, np.float32)
    shape = np.asarray(shape, np.float32)
    scale = np.asarray(scale, np.float32)
    loc = np.asarray(loc, np.float32)
    sp = int(start_pos)

    phi = _toeplitz_profile(m2)
    fold_mask = phi is not None
    kept = _kept_tiles(m2)
    kept_key = _banded_kept(kept, shape, scale, loc)

    LAST_RUN_INFO["build_args"] = (kept_key, fold_mask)
    nc = _build_program(kept_key, fold_mask)

    bf = ml_dtypes.bfloat16
    xT = np.ascontiguousarray(x32.T).astype(bf)
    inv_s = np.float32(1.0 / math.sqrt(HD))
    maskT = None if fold_mask else np.ascontiguousarray(m2.T)

    in_maps = []
    for c in range(N_CORES):
        heads = HEADS_OF_CORE[c]
        cols = np.concatenate(
            [np.arange(h * HD, (h + 1) * HD) for h in heads]
        )
        im = {
            "xT": xT,
            "wq": np.ascontiguousarray(wq32[:, cols] * inv_s).astype(bf),
            "wk": np.ascontiguousarray(wk32[:, cols]).astype(bf),
            "wv": np.ascontiguousarray(wv32[:, cols]).astype(bf),
            "wo": np.ascontiguousarray(wo32[cols, :]).astype(bf),
            "g": np.stack(
                [_eg_table(h, shape, scale, loc, sp, phi) for h in heads]
            ),
        }
        if not fold_mask:
            im["maskT"] = maskT
        in_maps.append(im)
    return nc, in_maps


def _reduce(results):
    acc = results[0]["outT"].astype(np.float32)
    for c in range(1, N_CORES):
        acc = acc + results[c]["outT"].astype(np.float32)
    return np.ascontiguousarray(acc.T)[None].astype(np.float32)


_RUNNER_CACHE = {}


def _get_runner(nc):
    """Build (once) a reusable jitted 8-core runner for the program `nc`.
    Mirrors bass2jax.run_bass_via_pjrt's multi-core path without output
    donation (outT is fully written by the kernel) so it can be re-invoked."""
    if id(nc) in _RUNNER_CACHE:
        return _RUNNER_CACHE[id(nc)]

    import jax
    from jax.sharding import Mesh, NamedSharding, PartitionSpec

    from jax.experimental.shard_map import shard_map
    from concourse import mybir
    from concourse.bass2jax import (
        _bass_exec_p,
        install_neuronx_cc_hook,
        partition_id_tensor,
    )

    install_neuronx_cc_hook()
    partition_name = nc.partition_id_tensor.name if nc.partition_id_tensor else None

    in_names, out_names, out_avals = [], [], []
    for alloc in nc.m.functions[0].allocations:
        if not isinstance(alloc, mybir.MemoryLocationSet):
            continue
        name = alloc.memorylocations[0].name
        if alloc.kind == "ExternalInput":
            if name != partition_name:
                in_names.append(name)
        elif alloc.kind == "ExternalOutput":
            out_names.append(name)
            out_avals.append(
                jax.core.ShapedArray(
                    tuple(alloc.tensor_shape), mybir.dt.np(alloc.dtype)
                )
            )
    n_params = len(in_names)
    all_names = in_names + out_names
    if partition_name is not None:
        all_names = all_names + [partition_name]

    def _body(*args):
        operands = list(args)
        if partition_name is not None:
            operands.append(partition_id_tensor())
        return tuple(
            _bass_exec_p.bind(
                *operands,
                out_avals=tuple(out_avals),
                in_names=tuple(all_names),
                out_names=tuple(out_names),
                lowering_input_output_aliases=(),
                sim_require_finite=True,
                sim_require_nnan=True,
                nc=nc,
            )
        )

    devices = jax.devices()[:N_CORES]
    mesh = Mesh(np.asarray(devices), ("core",))
    sharded = jax.jit(
        shard_map(
            _body,
            mesh=mesh,
            in_specs=(PartitionSpec("core"),) * (n_params + len(out_names)),
            out_specs=(PartitionSpec("core"),) * len(out_names),
            check_rep=False,
        ),
        keep_unused=True,
    )
    sh = NamedSharding(mesh, PartitionSpec("core"))

    def run(in_maps):
        concat_in = [
            np.concatenate(
                [np.asarray(in_maps[c][nm]) for c in range(N_CORES)], axis=0
            )
            for nm in in_names
        ]
        concat_zero = [
            np.zeros((N_CORES * av.shape[0], *av.shape[1:]), av.dtype)
            for av in out_avals
        ]
        dev_args = [jax.device_put(a, sh) for a in concat_in + concat_zero]
        out_arrs = sharded(*dev_args)
        return [
            {
                nm: np.asarray(out_arrs[i]).reshape(
                    N_CORES, *out_avals[i].shape
                )[c]
                for i, nm in enumerate(out_names)
            }
            for c in range(N_CORES)
        ]

    _RUNNER_CACHE[id(nc)] = run
    return run


def kernel(x, mask, wq, wk, wv, wo, shape, scale, loc, start_pos):
    nc, in_maps = prepare(x, mask, wq, wk, wv, wo, shape, scale, loc, start_pos)

    if os.environ.get("KBA_SIM", "0") == "1":
        from concourse import bass_interp

        n_sim = int(os.environ.get("KBA_SIM_CORES", str(N_CORES)))
        sim = bass_interp.MultiCoreSim(nc, n_sim)
        for c in range(n_sim):
            for k, v in in_maps[c].items():
                sim.cores[c].tensor(k)[:] = v
        sim.simulate()
        results = [
            {"outT": np.array(sim.cores[c].tensor("outT"))}
            for c in range(n_sim)
        ] + [
            {"outT": np.zeros((S, S), ml_dtypes.bfloat16)}
            for _ in range(N_CORES - n_sim)
        ]
        LAST_RUN_INFO["exec_time_ns"] = None
    else:
        results = _get_runner(nc)(in_maps)
        LAST_RUN_INFO["exec_time_ns"] = None

    LAST_RUN_INFO["results"] = results
    return _reduce(results)


# revision 25
# speedup vs baseline: 1.0634x; 1.0634x over previous
"""BayesianAttention (power-law prior + causal mask) on 8 trn2 cores.

Self-contained: builds a Bass/Tile kernel, shards heads across 8 NeuronCores
(2 heads per core; wq/wk/wv column-sharded, wo row-sharded), runs via a
jitted 8-core PJRT runner, and reduces the partial outputs on host.

Device-side layout is fully transposed (contraction dims on partitions) and
all matmul operands are bf16 (f32 PSUM accumulation):
  host sends x^T [c, i] bf16; device computes q^T/k^T/v^T = W^T x^T,
  transposes v, s^T[j,i] = k^T_j . q^T_i (raw, no bias),
  probs = exp(s^T) * EG, where EG = exp(prior + causal mask) is a
  precomputed per-head Toeplitz table ([128, 4096] bf16, masked entries
  exactly 0), o^T[d,i] = v^T probs / (w^T probs), out^T[e,i] = wo^T o^T.
Host returns sum_c(out^T_c)^T.

The kernel is emitted as one software-pipelined loop over 512-wide query
blocks: projections(ib) -> attention(ib, both heads) -> output(ib), which
keeps the tensor engine continuously busy (PSUM pool backpressure paces the
phases); x / out move as a few large batched DMAs on the two HWDGE queues.
"""

import math
import os

import ml_dtypes
import numpy as np

S = 2048          # sequence length
DIM = 2048        # model dim
H = 16            # heads
HD = 128          # head dim
N_CORES = 8
HL = H // N_CORES  # heads per core (2)
DL = HL * HD       # local projected dim (256)
IB = 512           # i-block (query block, moving free dim)
NIB = S // IB
NJT = S // 128     # key tiles of 128
GW = 4096          # EG table width (needs >= S + IB)
EPS = 1e-5
MASKED_THRESH = -1e8   # additive mask values below this mean "fully masked"

TRACE = bool(int(os.environ.get("KBA_TRACE", "0")))

LAG = 6  # scores->o-matmul emission lag (bounds live probs tiles)

LAST_RUN_INFO = {}

MM_DTYPE = "bf16"  # kept for test.py printout


# ---------------------------------------------------------------- tile patch
def _apply_tile_patch():
    """walrus CoreV3 codegen tolerates only one sync-wait on an InstDrain;
    the tile-exit drain waits on the whole global clock. Spread the waits
    across extra SP nops."""
    import concourse.tile as tile
    from concourse import mybir
    from concourse.vector_clock import ScopedClock

    if getattr(tile.TileContext, "_kba_patched", False):
        return

    def _drain_and_barrier(self, tick_clock, wait_clock):
        nc = self.nc
        drain_inst = nc.sync.drain()
        wait_clock.add_sem_waits(
            drain_inst.ins, ScopedClock({None: tick_clock.global_clock})
        )
        si = drain_inst.ins.sync_info
        waits = list(si.on_wait or [])
        if len(waits) > 1:
            si.on_wait = waits[:1]
            for i in range(1, len(waits)):
                nop = nc.sync.nop(nofuse=True)
                nop.ins.sync_info = mybir.SyncInfo(
                    on_wait=waits[i : i + 1], on_update=[]
                )
        nc.all_engine_barrier()
        assert self.sems is not None
        popped = nc._tile_sem_poison_stack.pop()
        assert popped is self._sem_poison
        nc.clear_and_free_semaphores(list(self.sems.allocated().values()))
        nc.all_engine_barrier()

    tile.TileContext._drain_and_barrier = _drain_and_barrier
    tile.TileContext._kba_patched = True

    try:
        import concourse.tile_utils as tile_utils

        tile_utils.max_sbuf_usage = 208 * 1024
    except Exception:
        pass


# ------------------------------------------------------------- host helpers
def _toeplitz_profile(m2):
    """If mask[i, j] == phi(j - i) for all i,j, return phi (length 2S-1,
    index t + S - 1), else None."""
    phi = np.empty(2 * S - 1, dtype=np.float32)
    phi[S - 1 :] = m2[0, :]
    phi[: S - 1] = m2[1:, 0][::-1]
    idx = (np.arange(S)[None, :] - np.arange(S)[:, None]) + (S - 1)
    if np.array_equal(phi[idx], m2):
        return phi
    return None


def _eg_table(head, shape, scale, loc, start_pos, phi):
    """[128, GW] bf16: EG[p, u] = exp(prior(d) + phi(-d)), where
    d = i - j = u - p - (S - 1). Masked / out-of-range entries are 0."""
    p = np.arange(128, dtype=np.int64)[:, None]
    u = np.arange(GW, dtype=np.int64)[None, :]
    d = u - p - (S - 1)          # i - j
    dist = (-d - start_pos).astype(np.float32)  # k_pos - q_pos
    sh = np.float32(shape[0, head, 0, 0])
    sc = np.float32(scale[0, head, 0, 0])
    lo = np.float32(loc[0, head, 0, 0])
    loc_t = np.float32(np.exp(lo) - np.exp(-lo))
    z = (dist - loc_t) * np.exp(sc, dtype=np.float32)
    g = -np.power(np.abs(z) + np.float32(EPS), sh, dtype=np.float32)
    if phi is not None:
        t = np.clip(-d + (S - 1), 0, 2 * S - 2)
        g = g + phi[t]
        g[(-d < -(S - 1)) | (-d > (S - 1))] = -np.inf  # out of range: exp -> 0
    with np.errstate(over="ignore", under="ignore"):
        eg = np.exp(g, dtype=np.float32)
    return np.ascontiguousarray(eg.astype(ml_dtypes.bfloat16))


def _kept_tiles(m2):
    """kept[ib] = list of key-tile indices jt whose [128 x IB] block is not
    fully masked. Must be identical for every core (single SPMD program)."""
    kept = []
    for ib in range(NIB):
        row = []
        for jt in range(NJT):
            blk = m2[ib * IB : (ib + 1) * IB, jt * 128 : (jt + 1) * 128]
            if blk.max() > MASKED_THRESH:
                row.append(jt)
        kept.append(row)
    return kept


# ------------------------------------------------------------ program build
_PROGRAM_CACHE = {}


def _build_program(kept_key, fold_mask, repeat=1):
    key = (kept_key, fold_mask, repeat)
    if key in _PROGRAM_CACHE:
        return _PROGRAM_CACHE[key]

    import concourse.bass as bass
    import concourse.tile as tile
    from concourse import bacc, mybir
    from concourse.masks import make_identity

    _apply_tile_patch()

    f32 = mybir.dt.float32
    bf16 = mybir.dt.bfloat16

    kept = [list(row) for row in kept_key]

    nc = bacc.Bacc(
        "TRN2", target_bir_lowering=False, debug=False, num_devices=N_CORES
    )
    xT_d = nc.dram_tensor("xT", [S, S], bf16, kind="ExternalInput")
    wq_d = nc.dram_tensor("wq", [S, DL], bf16, kind="ExternalInput")
    wk_d = nc.dram_tensor("wk", [S, DL], bf16, kind="ExternalInput")
    wv_d = nc.dram_tensor("wv", [S, DL], bf16, kind="ExternalInput")
    wo_d = nc.dram_tensor("wo", [DL, S], bf16, kind="ExternalInput")
    g_d = nc.dram_tensor("g", [HL, 128, GW], bf16, kind="ExternalInput")
    if not fold_mask:
        maskT_d = nc.dram_tensor("maskT", [S, S], f32, kind="ExternalInput")
    outT_d = nc.dram_tensor("outT", [S, S], bf16, kind="ExternalOutput")

    Exp = mybir.ActivationFunctionType.Exp
    Copy = mybir.ActivationFunctionType.Copy

    with tile.TileContext(nc) as tc:
        import contextlib

        with contextlib.ExitStack() as ctx:
            consts = ctx.enter_context(tc.tile_pool(name="consts", bufs=1))
            persist = ctx.enter_context(tc.tile_pool(name="persist", bufs=1))
            xpool = ctx.enter_context(tc.tile_pool(name="xp", bufs=2))
            qpool = ctx.enter_context(tc.tile_pool(name="qp", bufs=2))
            vtpool = ctx.enter_context(tc.tile_pool(name="vt", bufs=2))
            pbpool = ctx.enter_context(tc.tile_pool(name="pb", bufs=3))
            prpool = ctx.enter_context(tc.tile_pool(name="pr", bufs=LAG + 2))
            rpool = ctx.enter_context(tc.tile_pool(name="rp", bufs=2))
            ospool = ctx.enter_context(tc.tile_pool(name="os", bufs=2))
            obpool = ctx.enter_context(tc.tile_pool(name="ob", bufs=2))
            mpool = (
                ctx.enter_context(tc.tile_pool(name="mk", bufs=3))
                if not fold_mask
                else None
            )
            # PSUM: "big" (6 banks) round-robins phase-1 projection
            # accumulators (4+2), phase-2 scores tiles, and phase-3 output
            # tiles. "acc" (2 banks) serves v-transposes (phase 1) and the
            # o/softmax-sum accumulators (phase 2).
            big = ctx.enter_context(tc.tile_pool(name="big", bufs=6, space="PSUM"))
            acc = ctx.enter_context(tc.tile_pool(name="acc", bufs=2, space="PSUM"))

            # ---- constants / weights (batched DMAs, bf16 direct) ----
            # Split w into chunk-halves ordered by first use so the first
            # projection matmuls start ~4us earlier.
            wq_sb = consts.tile([128, NJT, DL], bf16, name="wq")
            wk_sb = consts.tile([128, NJT, DL], bf16, name="wk")
            wv_sb = consts.tile([128, NJT, DL], bf16, name="wv")
            wo_sb = consts.tile([128, HL, S], bf16, name="wo")
            g_sb = consts.tile([128, HL, GW], bf16, name="g")
            hc = NJT // 2
            for w_sb, w_d in ((wq_sb, wq_d), (wk_sb, wk_d)):
                nc.scalar.dma_start(
                    out=w_sb[:, :hc, :],
                    in_=w_d.ap()[: hc * 128, :].rearrange("(c p) d -> p c d", p=128),
                )
            for w_sb, w_d in ((wq_sb, wq_d), (wk_sb, wk_d)):
                nc.scalar.dma_start(
                    out=w_sb[:, hc:, :],
                    in_=w_d.ap()[hc * 128 :, :].rearrange("(c p) d -> p c d", p=128),
                )
            nc.scalar.dma_start(
                out=wv_sb[:], in_=wv_d.ap().rearrange("(c p) d -> p c d", p=128)
            )
            nc.scalar.dma_start(
                out=wo_sb[:], in_=wo_d.ap().rearrange("(h p) e -> p h e", p=128)
            )
            nc.scalar.dma_start(
                out=g_sb[:], in_=g_d.ap().rearrange("h p u -> p h u")
            )
            # [128, 128] all-ones stationary: the softmax-sum matmul then
            # produces Sum broadcast across all 128 partitions at no extra
            # PE cost (cycles scale with the moving width, not stationary m).
            ones_sb = consts.tile([128, 128], bf16, name="ones_sb")
            nc.vector.memset(ones_sb[:], 1.0)
            ident = consts.tile([128, 128], bf16)
            make_identity(nc, ident[:])

            kT = persist.tile([128, HL, S], bf16)          # [d, h, j]
            v_sb = persist.tile([128, HL, NJT, HD], bf16)  # [j, h, jt, d]

            qTb_of = {}
            o_sb_of = {}
            xt_of = {}

            def emit_xt_dma(ib):
                """Prefetch x^T for i-block ib (issued a pipeline step ahead
                of its pass-A consumers so the transfer is fully hidden)."""
                isl = bass.ts(ib, IB)
                xt = xpool.tile([128, NJT, IB], bf16, name="xt")
                xt_of[ib] = xt
                half = NJT // 2
                nc.sync.dma_start(
                    out=xt[:, :half, :],
                    in_=xT_d.ap()[: half * 128, isl].rearrange(
                        "(c p) i -> p c i", p=128
                    ),
                )
                nc.sync.dma_start(
                    out=xt[:, half:, :],
                    in_=xT_d.ap()[half * 128 :, isl].rearrange(
                        "(c p) i -> p c i", p=128
                    ),
                )

            def phase1_passA_units(ib):
                """Generator of pass-A work units for i-block ib: q^T/k^T
                projection matmuls per contraction chunk, then the qTb/kT
                copies. Yield granularity ~1.3us of PE work so units can be
                interleaved into phase 2 as PE filler."""
                isl = bass.ts(ib, IB)
                xt = xt_of[ib]
                qk_ps = {}
                for proj in range(2):
                    for dt_i in range(HL):
                        qk_ps[(proj, dt_i)] = big.tile(
                            [128, IB], f32, tag="big", name=f"ps{proj}{dt_i}"
                        )
                for ct in range(NJT):
                    for proj, w_sb in enumerate((wq_sb, wk_sb)):
                        for dt_i in range(HL):
                            nc.tensor.matmul(
                                qk_ps[(proj, dt_i)][:],
                                lhsT=w_sb[:, ct, dt_i * HD : (dt_i + 1) * HD],
                                rhs=xt[:, ct, :],
                                start=(ct == 0),
                                stop=(ct == NJT - 1),
                            )
                    yield
                qTb = qpool.tile([128, HL, IB], bf16, name="qTb")
                qTb_of[ib] = qTb
                for dt_i in range(HL):
                    nc.scalar.activation(qTb[:, dt_i, :], qk_ps[(0, dt_i)][:], Copy)
                    nc.vector.tensor_copy(kT[:, dt_i, isl], qk_ps[(1, dt_i)][:])
                yield

            def emit_phase1_passB(ib):
                """v^T projection + v transpose into v_sb for i-block ib.
                Emitted between phase2(prev) and phase3(prev) to fill the
                softmax-tail join with independent PE work."""
                xt = xt_of.pop(ib)
                v_ps = [
                    big.tile([128, IB], f32, tag="big", name=f"vps{dt_i}")
                    for dt_i in range(HL)
                ]
                for ct in range(NJT):
                    for dt_i in range(HL):
                        nc.tensor.matmul(
                            v_ps[dt_i][:],
                            lhsT=wv_sb[:, ct, dt_i * HD : (dt_i + 1) * HD],
                            rhs=xt[:, ct, :],
                            start=(ct == 0),
                            stop=(ct == NJT - 1),
                        )
                for dt_i in range(HL):
                    vt = vtpool.tile([128, IB], bf16, name="vt")
                    nc.scalar.activation(vt[:], v_ps[dt_i][:], Copy)
                    # transpose v^T [d, j] -> v [j, d] in 128-blocks
                    for s4 in range(IB // 128):
                        jt = (ib * IB) // 128 + s4
                        tp = acc.tile([128, 128], bf16, tag="acc", name="tp")
                        nc.tensor.transpose(
                            tp[:], vt[:, s4 * 128 : (s4 + 1) * 128], ident[:]
                        )
                        nc.vector.tensor_copy(v_sb[:, dt_i, jt, :], tp[:])

            def emit_phase2(ib, filler=None):
                """Attention for both local heads of i-block ib. `filler` is
                an optional iterator of next-block pass-A units, drained
                evenly across the score tiles to give PE independent work
                while the exp/multiply chain catches up."""
                isl = bass.ts(ib, IB)
                jts = kept[ib]
                qTb = qTb_of[ib]
                o_sb = ospool.tile([128, HL, IB], bf16, name="o_sb")
                o_sb_of[ib] = o_sb
                n_tiles = HL * len(jts)
                tiles_done = 0
                filler_done = 0
                filler_total = NJT + 1  # passA yields per ct chunk + copies

                def drain_filler():
                    nonlocal filler_done
                    if filler is None:
                        return
                    while filler_done / filler_total <= tiles_done / n_tiles:
                        try:
                            next(filler)
                            filler_done += 1
                        except StopIteration:
                            break
                # Causal narrowing: a diagonal key tile jt only affects
                # queries i >= jt*128, i.e. moving columns [off:], with
                # off = jt*128 - ib*IB. Skipped columns stay correct: the
                # first tile (off 0) writes the accumulators full-width, and
                # no later tile contributes to columns below its own off.
                offs = [max(0, jt * 128 - ib * IB) for jt in jts]
                if offs[0] != 0:  # first tile must init accumulators fully
                    offs = [0] * len(jts)
                for h in range(HL):
                    oacc = acc.tile([128, IB], f32, tag="acc", name="oacc")
                    sacc = acc.tile([128, IB], f32, tag="acc", name="sacc")
                    probs = {}

                    def emit_pv(idx):
                        jt = jts[idx]
                        off = offs[idx]
                        nc.tensor.matmul(
                            oacc[:, off:],
                            lhsT=v_sb[:, h, jt, :],
                            rhs=probs[idx][:, off:],
                            start=(idx == 0),
                            stop=(idx == len(jts) - 1),
                        )
                        nc.tensor.matmul(
                            sacc[:, off:],
                            lhsT=ones_sb[:],
                            rhs=probs[idx][:, off:],
                            start=(idx == 0),
                            stop=(idx == len(jts) - 1),
                        )

                    for idx, jt in enumerate(jts):
                        drain_filler()
                        tiles_done += 1
                        off = offs[idx]
                        sc = big.tile([128, IB], f32, tag="big", name="sc")
                        nc.tensor.matmul(
                            sc[:, off:],
                            lhsT=kT[:, h, jt * 128 : (jt + 1) * 128],
                            rhs=qTb[:, h, off:],
                            start=True,
                            stop=True,
                        )
                        if not fold_mask:
                            mt = mpool.tile([128, IB], f32)
                            nc.sync.dma_start(
                                out=mt[:],
                                in_=maskT_d[jt * 128 : (jt + 1) * 128, isl],
                            )
                            nc.vector.tensor_add(
                                sc[:, off:], sc[:, off:], mt[:, off:]
                            )
                        pb = pbpool.tile([128, IB], bf16, name="pb")
                        nc.scalar.activation(pb[:, off:], sc[:, off:], Exp)
                        base = ib * IB - jt * 128 + (S - 1)
                        pr = prpool.tile([128, IB], bf16, name="pr")
                        nc.vector.tensor_mul(
                            pr[:, off:],
                            pb[:, off:],
                            g_sb[:, h, base + off : base + IB],
                        )
                        probs[idx] = pr
                        if idx - LAG >= 0:
                            emit_pv(idx - LAG)
                    for idx in range(max(0, len(jts) - LAG), len(jts)):
                        emit_pv(idx)

                    rbc = rpool.tile([128, IB], f32, tag="rbc", name="rbc")
                    nc.vector.reciprocal(rbc[:], sacc[:])
                    nc.vector.tensor_mul(o_sb[:, h, :], oacc[:], rbc[:])
                if filler is not None:
                    for _ in filler:
                        pass

            def emit_phase3(ib):
                """out^T = wo^T o^T for i-block ib (partial; host sums cores).
                Output DMA goes out in two halves so the transfer overlaps the
                remaining copies."""
                isl = bass.ts(ib, IB)
                o_sb = o_sb_of.pop(ib)
                qTb_of.pop(ib, None)
                ob = obpool.tile([128, NJT, IB], bf16, name="ob")
                halves = (range(0, NJT // 2), range(NJT // 2, NJT))
                for hi, ets in enumerate(halves):
                    for et in ets:
                        po = big.tile([128, IB], f32, tag="big", name="po")
                        for h in range(HL):
                            nc.tensor.matmul(
                                po[:],
                                lhsT=wo_sb[:, h, et * 128 : (et + 1) * 128],
                                rhs=o_sb[:, h, :],
                                start=(h == 0),
                                stop=(h == HL - 1),
                            )
                        if et % 2 == 0:
                            nc.scalar.activation(ob[:, et, :], po[:], Copy)
                        else:
                            nc.vector.tensor_copy(ob[:, et, :], po[:])
                    e0, e1 = ets[0], ets[-1] + 1
                    nc.sync.dma_start(
                        out=outT_d.ap()[e0 * 128 : e1 * 128, isl].rearrange(
                            "(e p) i -> p e i", p=128
                        ),
                        in_=ob[:, e0:e1, :],
                    )

            # Software-pipelined schedule: the next block's pass-A projection
            # units are interleaved INTO attention(ib) as PE filler, its
            # v-projection (pass B) fills the softmax-tail -> output join,
            # and x prefetches run a full step ahead.
            blocks = [ib for _rep in range(repeat) for ib in range(NIB)]
            emit_xt_dma(blocks[0])
            for _ in phase1_passA_units(blocks[0]):
                pass
            if len(blocks) > 1:
                emit_xt_dma(blocks[1])
            emit_phase1_passB(blocks[0])
            for n, ib in enumerate(blocks):
                nxt = blocks[n + 1] if n + 1 < len(blocks) else None
                emit_phase2(
                    ib,
                    filler=phase1_passA_units(nxt) if nxt is not None else None,
                )
                if nxt is not None:
                    emit_phase1_passB(nxt)
                if n + 2 < len(blocks):
                    emit_xt_dma(blocks[n + 2])
                emit_phase3(ib)

    nc.compile()
    _PROGRAM_CACHE[key] = nc
    return nc


# ------------------------------------------------------------------- kernel
def prepare(x, mask, wq, wk, wv, wo, shape, scale, loc, start_pos):
    """Host prep: build/cache program and per-core input maps."""
    x32 = np.asarray(x, np.float32).reshape(S, DIM)
    m2 = np.asarray(mask, np.float32).reshape(S, S)
    wq32 = np.asarray(wq, np.float32)
    wk32 = np.asarray(wk, np.float32)
    wv32 = np.asarray(wv, np.float32)
    wo32 = np.asarray(wo, np.float32)
    shape = np.asarray(shape, np.float32)
    scale = np.asarray(scale, np.float32)
    loc = np.asarray(loc, np.float32)
    sp = int(start_pos)

    phi = _toeplitz_profile(m2)
    fold_mask = phi is not None
    kept = _kept_tiles(m2)
    kept_key = tuple(tuple(row) for row in kept)

    LAST_RUN_INFO["build_args"] = (kept_key, fold_mask)
    nc = _build_program(kept_key, fold_mask)

    bf = ml_dtypes.bfloat16
    xT = np.ascontiguousarray(x32.T).astype(bf)
    inv_s = np.float32(1.0 / math.sqrt(HD))
    maskT = None if fold_mask else np.ascontiguousarray(m2.T)

    in_maps = []
    for c in range(N_CORES):
        sl = slice(c * DL, (c + 1) * DL)
        im = {
            "xT": xT,
            "wq": np.ascontiguousarray(wq32[:, sl] * inv_s).astype(bf),
            "wk": np.ascontiguousarray(wk32[:, sl]).astype(bf),
            "wv": np.ascontiguousarray(wv32[:, sl]).astype(bf),
            "wo": np.ascontiguousarray(wo32[sl, :]).astype(bf),
            "g": np.stack(
                [
                    _eg_table(c * HL + h, shape, scale, loc, sp, phi)
                    for h in range(HL)
                ]
            ),
        }
        if not fold_mask:
            im["maskT"] = maskT
        in_maps.append(im)
    return nc, in_maps


def _reduce(results):
    acc = results[0]["outT"].astype(np.float32)
    for c in range(1, N_CORES):
        acc = acc + results[c]["outT"].astype(np.float32)
    return np.ascontiguousarray(acc.T)[None].astype(np.float32)


_RUNNER_CACHE = {}


def _get_runner(nc):
    """Build (once) a reusable jitted 8-core runner for the program `nc`.
    Mirrors bass2jax.run_bass_via_pjrt's multi-core path without output
    donation (outT is fully written by the kernel) so it can be re-invoked."""
    if id(nc) in _RUNNER_CACHE:
        return _RUNNER_CACHE[id(nc)]

    import jax
    from jax.sharding import Mesh, NamedSharding, PartitionSpec

    from jax.experimental.shard_map import shard_map
    from concourse import mybir
    from concourse.bass2jax import (
        _bass_exec_p,
        install_neuronx_cc_hook,
        partition_id_tensor,
    )

    install_neuronx_cc_hook()
    partition_name = nc.partition_id_tensor.name if nc.partition_id_tensor else None

    in_names, out_names, out_avals = [], [], []
    for alloc in nc.m.functions[0].allocations:
        if not isinstance(alloc, mybir.MemoryLocationSet):
            continue
        name = alloc.memorylocations[0].name
        if alloc.kind == "ExternalInput":
            if name != partition_name:
                in_names.append(name)
        elif alloc.kind == "ExternalOutput":
            out_names.append(name)
            out_avals.append(
                jax.core.ShapedArray(
                    tuple(alloc.tensor_shape), mybir.dt.np(alloc.dtype)
                )
            )
    n_params = len(in_names)
    all_names = in_names + out_names
    if partition_name is not None:
        all_names = all_names + [partition_name]

    def _body(*args):
        operands = list(args)
        if partition_name is not None:
            operands.append(partition_id_tensor())
        return tuple(
            _bass_exec_p.bind(
                *operands,
                out_avals=tuple(out_avals),
                in_names=tuple(all_names),
                out_names=tuple(out_names),
                lowering_input_output_aliases=(),
                sim_require_finite=True,
                sim_require_nnan=True,
                nc=nc,
            )
        )

    devices = jax.devices()[:N_CORES]
    mesh = Mesh(np.asarray(devices), ("core",))
    sharded = jax.jit(
        shard_map(
            _body,
            mesh=mesh,
            in_specs=(PartitionSpec("core"),) * (n_params + len(out_names)),
            out_specs=(PartitionSpec("core"),) * len(out_names),
            check_rep=False,
        ),
        keep_unused=True,
    )
    sh = NamedSharding(mesh, PartitionSpec("core"))

    def run(in_maps):
        concat_in = [
            np.concatenate(
                [np.asarray(in_maps[c][nm]) for c in range(N_CORES)], axis=0
            )
            for nm in in_names
        ]
        concat_zero = [
            np.zeros((N_CORES * av.shape[0], *av.shape[1:]), av.dtype)
            for av in out_avals
        ]
        dev_args = [jax.device_put(a, sh) for a in concat_in + concat_zero]
        out_arrs = sharded(*dev_args)
        return [
            {
                nm: np.asarray(out_arrs[i]).reshape(
                    N_CORES, *out_avals[i].shape
                )[c]
                for i, nm in enumerate(out_names)
            }
            for c in range(N_CORES)
        ]

    _RUNNER_CACHE[id(nc)] = run
    return run


def kernel(x, mask, wq, wk, wv, wo, shape, scale, loc, start_pos):
    nc, in_maps = prepare(x, mask, wq, wk, wv, wo, shape, scale, loc, start_pos)

    if os.environ.get("KBA_SIM", "0") == "1":
        from concourse import bass_interp

        n_sim = int(os.environ.get("KBA_SIM_CORES", str(N_CORES)))
        sim = bass_interp.MultiCoreSim(nc, n_sim)
        for c in range(n_sim):
            for k, v in in_maps[c].items():
                sim.cores[c].tensor(k)[:] = v
        sim.simulate()
        results = [
            {"outT": np.array(sim.cores[c].tensor("outT"))}
            for c in range(n_sim)
        ] + [
            {"outT": np.zeros((S, S), ml_dtypes.bfloat16)}
            for _ in range(N_CORES - n_sim)
        ]
        LAST_RUN_INFO["exec_time_ns"] = None
    else:
        results = _get_runner(nc)(in_maps)
        LAST_RUN_INFO["exec_time_ns"] = None

    LAST_RUN_INFO["results"] = results
    return _reduce(results)


# revision 29
# speedup vs baseline: 1.7025x; 1.6010x over previous
"""BayesianAttention (power-law prior + causal mask) on 8 trn2 cores.

Self-contained: builds a Bass/Tile kernel, shards heads across 8 NeuronCores
(2 heads per core; wq/wk/wv column-sharded, wo row-sharded), runs via a
jitted 8-core PJRT runner, and reduces the partial outputs on host.

Device-side layout is fully transposed (contraction dims on partitions) and
all matmul operands are bf16 (f32 PSUM accumulation):
  host sends x^T [c, i] bf16; device computes q^T/k^T/v^T = W^T x^T,
  transposes v, s^T[j,i] = k^T_j . q^T_i (raw, no bias),
  probs = exp(s^T) * EG, where EG = exp(prior + causal mask) is a
  precomputed per-head Toeplitz table ([128, 4096] bf16, masked entries
  exactly 0), o^T[d,i] = v^T probs / (w^T probs), out^T[e,i] = wo^T o^T.
Host returns sum_c(out^T_c)^T.

The kernel is emitted as one software-pipelined loop over 512-wide query
blocks: projections(ib) -> attention(ib, both heads) -> output(ib), which
keeps the tensor engine continuously busy (PSUM pool backpressure paces the
phases); x / out move as a few large batched DMAs on the two HWDGE queues.
"""

import math
import os

import ml_dtypes
import numpy as np

S = 2048          # sequence length
DIM = 2048        # model dim
H = 16            # heads
HD = 128          # head dim
N_CORES = 8
HL = H // N_CORES  # heads per core (2)
DL = HL * HD       # local projected dim (256)
IB = 512           # i-block (query block, moving free dim)
NIB = S // IB
NJT = S // 128     # key tiles of 128
GW = 4096          # EG table width (needs >= S + IB)
EPS = 1e-5
MASKED_THRESH = -1e8   # additive mask values below this mean "fully masked"

TRACE = bool(int(os.environ.get("KBA_TRACE", "0")))

LAG = 6  # scores->o-matmul emission lag (bounds live probs tiles)

LAST_RUN_INFO = {}

MM_DTYPE = "bf16"  # kept for test.py printout


# ---------------------------------------------------------------- tile patch
def _apply_tile_patch():
    """walrus CoreV3 codegen tolerates only one sync-wait on an InstDrain;
    the tile-exit drain waits on the whole global clock. Spread the waits
    across extra SP nops."""
    import concourse.tile as tile
    from concourse import mybir
    from concourse.vector_clock import ScopedClock

    if getattr(tile.TileContext, "_kba_patched", False):
        return

    def _drain_and_barrier(self, tick_clock, wait_clock):
        nc = self.nc
        drain_inst = nc.sync.drain()
        wait_clock.add_sem_waits(
            drain_inst.ins, ScopedClock({None: tick_clock.global_clock})
        )
        si = drain_inst.ins.sync_info
        waits = list(si.on_wait or [])
        if len(waits) > 1:
            si.on_wait = waits[:1]
            for i in range(1, len(waits)):
                nop = nc.sync.nop(nofuse=True)
                nop.ins.sync_info = mybir.SyncInfo(
                    on_wait=waits[i : i + 1], on_update=[]
                )
        nc.all_engine_barrier()
        assert self.sems is not None
        popped = nc._tile_sem_poison_stack.pop()
        assert popped is self._sem_poison
        nc.clear_and_free_semaphores(list(self.sems.allocated().values()))
        nc.all_engine_barrier()

    tile.TileContext._drain_and_barrier = _drain_and_barrier
    tile.TileContext._kba_patched = True

    try:
        import concourse.tile_utils as tile_utils

        tile_utils.max_sbuf_usage = 208 * 1024
    except Exception:
        pass


# ------------------------------------------------------------- host helpers
def _toeplitz_profile(m2):
    """If mask[i, j] == phi(j - i) for all i,j, return phi (length 2S-1,
    index t + S - 1), else None."""
    phi = np.empty(2 * S - 1, dtype=np.float32)
    phi[S - 1 :] = m2[0, :]
    phi[: S - 1] = m2[1:, 0][::-1]
    idx = (np.arange(S)[None, :] - np.arange(S)[:, None]) + (S - 1)
    if np.array_equal(phi[idx], m2):
        return phi
    return None


def _eg_table(head, shape, scale, loc, start_pos, phi):
    """[128, GW] bf16: EG[p, u] = exp(prior(d) + phi(-d)), where
    d = i - j = u - p - (S - 1). Masked / out-of-range entries are 0."""
    p = np.arange(128, dtype=np.int64)[:, None]
    u = np.arange(GW, dtype=np.int64)[None, :]
    d = u - p - (S - 1)          # i - j
    dist = (-d - start_pos).astype(np.float32)  # k_pos - q_pos
    sh = np.float32(shape[0, head, 0, 0])
    sc = np.float32(scale[0, head, 0, 0])
    lo = np.float32(loc[0, head, 0, 0])
    loc_t = np.float32(np.exp(lo) - np.exp(-lo))
    z = (dist - loc_t) * np.exp(sc, dtype=np.float32)
    g = -np.power(np.abs(z) + np.float32(EPS), sh, dtype=np.float32)
    if phi is not None:
        t = np.clip(-d + (S - 1), 0, 2 * S - 2)
        g = g + phi[t]
        g[(-d < -(S - 1)) | (-d > (S - 1))] = -np.inf  # out of range: exp -> 0
    with np.errstate(over="ignore", under="ignore"):
        eg = np.exp(g, dtype=np.float32)
    return np.ascontiguousarray(eg.astype(ml_dtypes.bfloat16))


def _kept_tiles(m2):
    """kept[ib] = list of key-tile indices jt whose [128 x IB] block is not
    fully masked. Must be identical for every core (single SPMD program)."""
    kept = []
    for ib in range(NIB):
        row = []
        for jt in range(NJT):
            blk = m2[ib * IB : (ib + 1) * IB, jt * 128 : (jt + 1) * 128]
            if blk.max() > MASKED_THRESH:
                row.append(jt)
        kept.append(row)
    return kept


BAND_THR = 30.0  # drop key tiles once the prior bias is below -BAND_THR


def _prior_reach(head, shape, scale, loc):
    """Smallest distance d* such that the prior penalty is <= -BAND_THR for
    every d >= d* (attention weight < e^-30 of an undecayed key). 2*S if the
    prior never decays that far."""
    sh = np.float32(shape[0, head, 0, 0])
    sc = np.float32(scale[0, head, 0, 0])
    lo = np.float32(loc[0, head, 0, 0])
    loc_t = np.exp(lo) - np.exp(-lo)
    d = np.arange(2 * S, dtype=np.float32)
    pen = (np.abs(-d - loc_t) * np.exp(sc) + np.float32(EPS)) ** sh
    ok = pen >= BAND_THR
    if not ok.any():
        return 2 * S
    # first index from which ok holds for all larger d
    rev_ok = np.minimum.accumulate(ok[::-1])[::-1]
    idx = np.argmax(rev_ok)
    if not rev_ok[idx]:
        return 2 * S
    return int(idx)


HEADS_OF_CORE = [[c, c + H // 2] for c in range(N_CORES)]
# Slot s of every core runs the same instruction stream; slot 0 holds the
# steep-slope (short-reach) heads 0..7 so its key-tile band can be cut
# uniformly, slot 1 holds the flat heads 8..15 (full causal reach).


def _banded_kept(kept, shape, scale, loc):
    """Per-slot kept-tile lists: drop tiles whose whole [IB x 128] block is
    beyond every covered head's prior reach. Identical across cores by
    construction (slot reach = max over cores)."""
    kept_slots = []
    for s in range(HL):
        reach = max(
            _prior_reach(HEADS_OF_CORE[c][s], shape, scale, loc)
            for c in range(N_CORES)
        )
        rows = []
        for ib in range(NIB):
            i0 = ib * IB
            rows.append(
                tuple(
                    jt for jt in kept[ib] if jt * 128 + 127 >= i0 - reach
                )
            )
        kept_slots.append(tuple(rows))
    return tuple(kept_slots)


# ------------------------------------------------------------ program build
_PROGRAM_CACHE = {}


def _build_program(kept_key, fold_mask, repeat=1):
    key = (kept_key, fold_mask, repeat)
    if key in _PROGRAM_CACHE:
        return _PROGRAM_CACHE[key]

    import concourse.bass as bass
    import concourse.tile as tile
    from concourse import bacc, mybir
    from concourse.masks import make_identity

    _apply_tile_patch()

    f32 = mybir.dt.float32
    bf16 = mybir.dt.bfloat16

    kept = [[list(row) for row in slot_rows] for slot_rows in kept_key]

    nc = bacc.Bacc(
        "TRN2", target_bir_lowering=False, debug=False, num_devices=N_CORES
    )
    xT_d = nc.dram_tensor("xT", [S, S], bf16, kind="ExternalInput")
    wq_d = nc.dram_tensor("wq", [S, DL], bf16, kind="ExternalInput")
    wk_d = nc.dram_tensor("wk", [S, DL], bf16, kind="ExternalInput")
    wv_d = nc.dram_tensor("wv", [S, DL], bf16, kind="ExternalInput")
    wo_d = nc.dram_tensor("wo", [DL, S], bf16, kind="ExternalInput")
    g_d = nc.dram_tensor("g", [HL, 128, GW], bf16, kind="ExternalInput")
    if not fold_mask:
        maskT_d = nc.dram_tensor("maskT", [S, S], f32, kind="ExternalInput")
    outT_d = nc.dram_tensor("outT", [S, S], bf16, kind="ExternalOutput")

    Exp = mybir.ActivationFunctionType.Exp
    Copy = mybir.ActivationFunctionType.Copy

    with tile.TileContext(nc) as tc:
        import contextlib

        with contextlib.ExitStack() as ctx:
            consts = ctx.enter_context(tc.tile_pool(name="consts", bufs=1))
            persist = ctx.enter_context(tc.tile_pool(name="persist", bufs=1))
            xpool = ctx.enter_context(tc.tile_pool(name="xp", bufs=2))
            qpool = ctx.enter_context(tc.tile_pool(name="qp", bufs=2))
            vtpool = ctx.enter_context(tc.tile_pool(name="vt", bufs=2))
            pbpool = ctx.enter_context(tc.tile_pool(name="pb", bufs=3))
            prpool = ctx.enter_context(tc.tile_pool(name="pr", bufs=LAG + 2))
            rpool = ctx.enter_context(tc.tile_pool(name="rp", bufs=2))
            ospool = ctx.enter_context(tc.tile_pool(name="os", bufs=2))
            obpool = ctx.enter_context(tc.tile_pool(name="ob", bufs=2))
            mpool = (
                ctx.enter_context(tc.tile_pool(name="mk", bufs=3))
                if not fold_mask
                else None
            )
            # PSUM: "big" (6 banks) round-robins phase-1 projection
            # accumulators (4+2), phase-2 scores tiles, and phase-3 output
            # tiles. "acc" (2 banks) serves v-transposes (phase 1) and the
            # o/softmax-sum accumulators (phase 2).
            big = ctx.enter_context(tc.tile_pool(name="big", bufs=6, space="PSUM"))
            acc = ctx.enter_context(tc.tile_pool(name="acc", bufs=2, space="PSUM"))

            # ---- constants / weights (batched DMAs, bf16 direct) ----
            # Split w into chunk-halves ordered by first use so the first
            # projection matmuls start ~4us earlier.
            wq_sb = consts.tile([128, NJT, DL], bf16, name="wq")
            wk_sb = consts.tile([128, NJT, DL], bf16, name="wk")
            wv_sb = consts.tile([128, NJT, DL], bf16, name="wv")
            wo_sb = consts.tile([128, HL, S], bf16, name="wo")
            g_sb = consts.tile([128, HL, GW], bf16, name="g")
            qc = NJT // 4
            for c0, c1 in ((0, qc), (qc, NJT // 2), (NJT // 2, NJT)):
                for w_sb, w_d in ((wq_sb, wq_d), (wk_sb, wk_d)):
                    nc.scalar.dma_start(
                        out=w_sb[:, c0:c1, :],
                        in_=w_d.ap()[c0 * 128 : c1 * 128, :].rearrange(
                            "(c p) d -> p c d", p=128
                        ),
                    )
            nc.scalar.dma_start(
                out=wv_sb[:], in_=wv_d.ap().rearrange("(c p) d -> p c d", p=128)
            )
            nc.scalar.dma_start(
                out=wo_sb[:], in_=wo_d.ap().rearrange("(h p) e -> p h e", p=128)
            )
            nc.scalar.dma_start(
                out=g_sb[:], in_=g_d.ap().rearrange("h p u -> p h u")
            )
            # [128, 128] all-ones stationary: the softmax-sum matmul then
            # produces Sum broadcast across all 128 partitions at no extra
            # PE cost (cycles scale with the moving width, not stationary m).
            ones_sb = consts.tile([128, 128], bf16, name="ones_sb")
            nc.vector.memset(ones_sb[:], 1.0)
            ident = consts.tile([128, 128], bf16)
            make_identity(nc, ident[:])

            kT = persist.tile([128, HL, S], bf16)          # [d, h, j]
            v_sb = persist.tile([128, HL, NJT, HD], bf16)  # [j, h, jt, d]

            qTb_of = {}
            o_sb_of = {}
            xt_of = {}

            def emit_xt_dma(ib, first=False):
                """Prefetch x^T for i-block ib (issued a pipeline step ahead
                of its pass-A consumers so the transfer is fully hidden).
                The prologue block streams in quarters so the very first
                projection matmuls can start ~4us earlier."""
                isl = bass.ts(ib, IB)
                xt = xpool.tile([128, NJT, IB], bf16, name="xt")
                xt_of[ib] = xt
                step = NJT // 4 if first else NJT // 2
                for c0 in range(0, NJT, step):
                    nc.sync.dma_start(
                        out=xt[:, c0 : c0 + step, :],
                        in_=xT_d.ap()[c0 * 128 : (c0 + step) * 128, isl].rearrange(
                            "(c p) i -> p c i", p=128
                        ),
                    )

            def phase1_passA_units(ib):
                """Generator of pass-A work units for i-block ib: q^T/k^T
                projection matmuls per contraction chunk, then the qTb/kT
                copies. Yield granularity ~1.3us of PE work so units can be
                interleaved into phase 2 as PE filler."""
                isl = bass.ts(ib, IB)
                xt = xt_of[ib]
                qk_ps = {}
                for proj in range(2):
                    for dt_i in range(HL):
                        qk_ps[(proj, dt_i)] = big.tile(
                            [128, IB], f32, tag="big", name=f"ps{proj}{dt_i}"
                        )
                for ct in range(NJT):
                    for proj, w_sb in enumerate((wq_sb, wk_sb)):
                        for dt_i in range(HL):
                            nc.tensor.matmul(
                                qk_ps[(proj, dt_i)][:],
                                lhsT=w_sb[:, ct, dt_i * HD : (dt_i + 1) * HD],
                                rhs=xt[:, ct, :],
                                start=(ct == 0),
                                stop=(ct == NJT - 1),
                            )
                    yield
                qTb = qpool.tile([128, HL, IB], bf16, name="qTb")
                qTb_of[ib] = qTb
                for dt_i in range(HL):
                    nc.scalar.activation(qTb[:, dt_i, :], qk_ps[(0, dt_i)][:], Copy)
                    nc.vector.tensor_copy(kT[:, dt_i, isl], qk_ps[(1, dt_i)][:])
                yield

            def emit_phase1_passB(ib):
                """v^T projection + v transpose into v_sb for i-block ib.
                Emitted between phase2(prev) and phase3(prev) to fill the
                softmax-tail join with independent PE work."""
                xt = xt_of.pop(ib)
                v_ps = [
                    big.tile([128, IB], f32, tag="big", name=f"vps{dt_i}")
                    for dt_i in range(HL)
                ]
                for ct in range(NJT):
                    for dt_i in range(HL):
                        nc.tensor.matmul(
                            v_ps[dt_i][:],
                            lhsT=wv_sb[:, ct, dt_i * HD : (dt_i + 1) * HD],
                            rhs=xt[:, ct, :],
                            start=(ct == 0),
                            stop=(ct == NJT - 1),
                        )
                for dt_i in range(HL):
                    vt = vtpool.tile([128, IB], bf16, name="vt")
                    nc.scalar.activation(vt[:], v_ps[dt_i][:], Copy)
                    # transpose v^T [d, j] -> v [j, d] in 128-blocks
                    for s4 in range(IB // 128):
                        jt = (ib * IB) // 128 + s4
                        tp = acc.tile([128, 128], bf16, tag="acc", name="tp")
                        nc.tensor.transpose(
                            tp[:], vt[:, s4 * 128 : (s4 + 1) * 128], ident[:]
                        )
                        nc.vector.tensor_copy(v_sb[:, dt_i, jt, :], tp[:])

            def emit_phase2(ib, filler=None):
                """Attention for both local heads of i-block ib. `filler` is
                an optional iterator of next-block pass-A units, drained
                evenly across the score tiles to give PE independent work
                while the exp/multiply chain catches up."""
                isl = bass.ts(ib, IB)
                qTb = qTb_of[ib]
                o_sb = ospool.tile([128, HL, IB], bf16, name="o_sb")
                o_sb_of[ib] = o_sb
                n_tiles = sum(len(kept[h][ib]) for h in range(HL))
                tiles_done = 0
                filler_done = 0
                filler_total = NJT + 1  # passA yields per ct chunk + copies

                def drain_filler():
                    nonlocal filler_done
                    if filler is None:
                        return
                    while filler_done / filler_total <= tiles_done / n_tiles:
                        try:
                            next(filler)
                            filler_done += 1
                        except StopIteration:
                            break
                for h in range(HL):
                    jts = kept[h][ib]
                    # Causal narrowing: a diagonal key tile jt only affects
                    # queries i >= jt*128, i.e. moving columns [off:], with
                    # off = jt*128 - ib*IB. Skipped columns stay correct: the
                    # first tile (off 0) writes the accumulators full-width,
                    # and no later tile contributes below its own off.
                    offs = [max(0, jt * 128 - ib * IB) for jt in jts]
                    if offs[0] != 0:  # first tile must init accums fully
                        offs = [0] * len(jts)
                    oacc = acc.tile([128, IB], f32, tag="acc", name="oacc")
                    sacc = acc.tile([128, IB], f32, tag="acc", name="sacc")
                    probs = {}

                    def emit_pv(idx):
                        jt = jts[idx]
                        off = offs[idx]
                        nc.tensor.matmul(
                            oacc[:, off:],
                            lhsT=v_sb[:, h, jt, :],
                            rhs=probs[idx][:, off:],
                            start=(idx == 0),
                            stop=(idx == len(jts) - 1),
                        )
                        nc.tensor.matmul(
                            sacc[:, off:],
                            lhsT=ones_sb[:],
                            rhs=probs[idx][:, off:],
                            start=(idx == 0),
                            stop=(idx == len(jts) - 1),
                        )

                    for idx, jt in enumerate(jts):
                        drain_filler()
                        tiles_done += 1
                        off = offs[idx]
                        sc = big.tile([128, IB], f32, tag="big", name="sc")
                        nc.tensor.matmul(
                            sc[:, off:],
                            lhsT=kT[:, h, jt * 128 : (jt + 1) * 128],
                            rhs=qTb[:, h, off:],
                            start=True,
                            stop=True,
                        )
                        if not fold_mask:
                            mt = mpool.tile([128, IB], f32)
                            nc.sync.dma_start(
                                out=mt[:],
                                in_=maskT_d[jt * 128 : (jt + 1) * 128, isl],
                            )
                            nc.vector.tensor_add(
                                sc[:, off:], sc[:, off:], mt[:, off:]
                            )
                        pb = pbpool.tile([128, IB], bf16, name="pb")
                        nc.scalar.activation(pb[:, off:], sc[:, off:], Exp)
                        base = ib * IB - jt * 128 + (S - 1)
                        pr = prpool.tile([128, IB], bf16, name="pr")
                        nc.vector.tensor_mul(
                            pr[:, off:],
                            pb[:, off:],
                            g_sb[:, h, base + off : base + IB],
                        )
                        probs[idx] = pr
                        if idx - LAG >= 0:
                            emit_pv(idx - LAG)
                    for idx in range(max(0, len(jts) - LAG), len(jts)):
                        emit_pv(idx)

                    rbc = rpool.tile([128, IB], f32, tag="rbc", name="rbc")
                    nc.vector.reciprocal(rbc[:], sacc[:])
                    nc.vector.tensor_mul(o_sb[:, h, :], oacc[:], rbc[:])
                if filler is not None:
                    for _ in filler:
                        pass

            def emit_phase3(ib):
                """out^T = wo^T o^T for i-block ib (partial; host sums cores).
                Output DMA goes out in two halves so the transfer overlaps the
                remaining copies."""
                isl = bass.ts(ib, IB)
                o_sb = o_sb_of.pop(ib)
                qTb_of.pop(ib, None)
                ob = obpool.tile([128, NJT, IB], bf16, name="ob")
                halves = (range(0, NJT // 2), range(NJT // 2, NJT))
                for hi, ets in enumerate(halves):
                    for et in ets:
                        po = big.tile([128, IB], f32, tag="big", name="po")
                        for h in range(HL):
                            nc.tensor.matmul(
                                po[:],
                                lhsT=wo_sb[:, h, et * 128 : (et + 1) * 128],
                                rhs=o_sb[:, h, :],
                                start=(h == 0),
                                stop=(h == HL - 1),
                            )
                        if et % 2 == 0:
                            nc.scalar.activation(ob[:, et, :], po[:], Copy)
                        else:
                            nc.vector.tensor_copy(ob[:, et, :], po[:])
                    e0, e1 = ets[0], ets[-1] + 1
                    nc.sync.dma_start(
                        out=outT_d.ap()[e0 * 128 : e1 * 128, isl].rearrange(
                            "(e p) i -> p e i", p=128
                        ),
                        in_=ob[:, e0:e1, :],
                    )

            # Software-pipelined schedule: the next block's pass-A projection
            # units are interleaved INTO attention(ib) as PE filler, its
            # v-projection (pass B) fills the softmax-tail -> output join,
            # and x prefetches run a full step ahead.
            blocks = [ib for _rep in range(repeat) for ib in range(NIB)]
            emit_xt_dma(blocks[0], first=True)
            for _ in phase1_passA_units(blocks[0]):
                pass
            if len(blocks) > 1:
                emit_xt_dma(blocks[1])
            emit_phase1_passB(blocks[0])
            for n, ib in enumerate(blocks):
                nxt = blocks[n + 1] if n + 1 < len(blocks) else None
                emit_phase2(
                    ib,
                    filler=phase1_passA_units(nxt) if nxt is not None else None,
                )
                if nxt is not None:
                    emit_phase1_passB(nxt)
                if n + 2 < len(blocks):
                    emit_xt_dma(blocks[n + 2])
                emit_phase3(ib)

    nc.compile()
    _PROGRAM_CACHE[key] = nc
    return nc


# ------------------------------------------------------------------- kernel
def prepare(x, mask, wq, wk, wv, wo, shape, scale, loc, start_pos):
    """Host prep: build/cache program and per-core input maps."""
    x32 = np.asarray(x, np.float32).reshape(S, DIM)
    m2 = np.asarray(mask, np.float32).reshape(S, S)
    wq32 = np.asarray(wq, np.float32)
    wk32 = np.asarray(wk, np.float32)
    wv32 = np.asarray(wv, np.float32)
    wo32 = np.asarray(wo, np.float32)
    shape = np.asarray(shape, np.float32)
    scale = np.asarray(scale, np.float32)
    loc = np.asarray(loc, np.float32)
    sp = int(start_pos)

    phi = _toeplitz_profile(m2)
    fold_mask = phi is not None
    kept = _kept_tiles(m2)
    kept_key = _banded_kept(kept, shape, scale, loc)

    LAST_RUN_INFO["build_args"] = (kept_key, fold_mask)
    nc = _build_program(kept_key, fold_mask)

    bf = ml_dtypes.bfloat16
    xT = np.ascontiguousarray(x32.T).astype(bf)
    inv_s = np.float32(1.0 / math.sqrt(HD))
    maskT = None if fold_mask else np.ascontiguousarray(m2.T)

    in_maps = []
    for c in range(N_CORES):
        heads = HEADS_OF_CORE[c]
        cols = np.concatenate(
            [np.arange(h * HD, (h + 1) * HD) for h in heads]
        )
        im = {
            "xT": xT,
            "wq": np.ascontiguousarray(wq32[:, cols] * inv_s).astype(bf),
            "wk": np.ascontiguousarray(wk32[:, cols]).astype(bf),
            "wv": np.ascontiguousarray(wv32[:, cols]).astype(bf),
            "wo": np.ascontiguousarray(wo32[cols, :]).astype(bf),
            "g": np.stack(
                [_eg_table(h, shape, scale, loc, sp, phi) for h in heads]
            ),
        }
        if not fold_mask:
            im["maskT"] = maskT
        in_maps.append(im)
    return nc, in_maps


def _reduce(results):
    acc = results[0]["outT"].astype(np.float32)
    for c in range(1, N_CORES):
        acc = acc + results[c]["outT"].astype(np.float32)
    return np.ascontiguousarray(acc.T)[None].astype(np.float32)


_RUNNER_CACHE = {}


def _get_runner(nc):
    """Build (once) a reusable jitted 8-core runner for the program `nc`.
    Mirrors bass2jax.run_bass_via_pjrt's multi-core path without output
    donation (outT is fully written by the kernel) so it can be re-invoked."""
    if id(nc) in _RUNNER_CACHE:
        return _RUNNER_CACHE[id(nc)]

    import jax
    from jax.sharding import Mesh, NamedSharding, PartitionSpec

    from jax.experimental.shard_map import shard_map
    from concourse import mybir
    from concourse.bass2jax import (
        _bass_exec_p,
        install_neuronx_cc_hook,
        partition_id_tensor,
    )

    install_neuronx_cc_hook()
    partition_name = nc.partition_id_tensor.name if nc.partition_id_tensor else None

    in_names, out_names, out_avals = [], [], []
    for alloc in nc.m.functions[0].allocations:
        if not isinstance(alloc, mybir.MemoryLocationSet):
            continue
        name = alloc.memorylocations[0].name
        if alloc.kind == "ExternalInput":
            if name != partition_name:
                in_names.append(name)
        elif alloc.kind == "ExternalOutput":
            out_names.append(name)
            out_avals.append(
                jax.core.ShapedArray(
                    tuple(alloc.tensor_shape), mybir.dt.np(alloc.dtype)
                )
            )
    n_params = len(in_names)
    all_names = in_names + out_names
    if partition_name is not None:
        all_names = all_names + [partition_name]

    def _body(*args):
        operands = list(args)
        if partition_name is not None:
            operands.append(partition_id_tensor())
        return tuple(
            _bass_exec_p.bind(
                *operands,
                out_avals=tuple(out_avals),
                in_names=tuple(all_names),
                out_names=tuple(out_names),
                lowering_input_output_aliases=(),
                sim_require_finite=True,
                sim_require_nnan=True,
                nc=nc,
            )
        )

    devices = jax.devices()[:N_CORES]
    mesh = Mesh(np.asarray(devices), ("core",))
    sharded = jax.jit(
        shard_map(
            _body,
            mesh=mesh,
            in_specs=(PartitionSpec("core"),) * (n_params + len(out_names)),
            out_specs=(PartitionSpec("core"),) * len(out_names),
            check_rep=False,
        ),
        keep_unused=True,
    )
    sh = NamedSharding(mesh, PartitionSpec("core"))

    def run(in_maps):
        concat_in = [
            np.concatenate(
                [np.asarray(in_maps[c][nm]) for c in range(N_CORES)], axis=0
            )
            for nm in in_names
        ]
        concat_zero = [
            np.zeros((N_CORES * av.shape[0], *av.shape[1:]), av.dtype)
            for av in out_avals
        ]
        dev_args = [jax.device_put(a, sh) for a in concat_in + concat_zero]
        out_arrs = sharded(*dev_args)
        return [
            {
                nm: np.asarray(out_arrs[i]).reshape(
                    N_CORES, *out_avals[i].shape
                )[c]
                for i, nm in enumerate(out_names)
            }
            for c in range(N_CORES)
        ]

    _RUNNER_CACHE[id(nc)] = run
    return run


def kernel(x, mask, wq, wk, wv, wo, shape, scale, loc, start_pos):
    nc, in_maps = prepare(x, mask, wq, wk, wv, wo, shape, scale, loc, start_pos)

    if os.environ.get("KBA_SIM", "0") == "1":
        from concourse import bass_interp

        n_sim = int(os.environ.get("KBA_SIM_CORES", str(N_CORES)))
        sim = bass_interp.MultiCoreSim(nc, n_sim)
        for c in range(n_sim):
            for k, v in in_maps[c].items():
                sim.cores[c].tensor(k)[:] = v
        sim.simulate()
        results = [
            {"outT": np.array(sim.cores[c].tensor("outT"))}
            for c in range(n_sim)
        ] + [
            {"outT": np.zeros((S, S), ml_dtypes.bfloat16)}
            for _ in range(N_CORES - n_sim)
        ]
        LAST_RUN_INFO["exec_time_ns"] = None
    else:
        results = _get_runner(nc)(in_maps)
        LAST_RUN_INFO["exec_time_ns"] = None

    LAST_RUN_INFO["results"] = results
    return _reduce(results)


# revision 35
# speedup vs baseline: 1.9002x; 1.1162x over previous
"""BayesianAttention (power-law prior + causal mask) on 8 trn2 cores.

Self-contained: builds a Bass/Tile kernel, shards heads across 8 NeuronCores
(2 heads per core; wq/wk/wv column-sharded, wo row-sharded), runs via a
jitted 8-core PJRT runner, and reduces the partial outputs on host.

Device-side layout is fully transposed (contraction dims on partitions) and
all matmul operands are bf16 (f32 PSUM accumulation):
  host sends x^T [c, i] bf16; device computes q^T/k^T/v^T = W^T x^T,
  transposes v, s^T[j,i] = k^T_j . q^T_i (raw, no bias),
  probs = exp(s^T) * EG, where EG = exp(prior + causal mask) is a
  precomputed per-head Toeplitz table ([128, 4096] bf16, masked entries
  exactly 0), o^T[d,i] = v^T probs / (w^T probs), out^T[e,i] = wo^T o^T.
Host returns sum_c(out^T_c)^T.

The kernel is emitted as one software-pipelined loop over 512-wide query
blocks: projections(ib) -> attention(ib, both heads) -> output(ib), which
keeps the tensor engine continuously busy (PSUM pool backpressure paces the
phases); x / out move as a few large batched DMAs on the two HWDGE queues.
"""

import math
import os

import ml_dtypes
import numpy as np

S = 2048          # sequence length
DIM = 2048        # model dim
H = 16            # heads
HD = 128          # head dim
N_CORES = 8
HL = H // N_CORES  # heads per core (2)
DL = HL * HD       # local projected dim (256)
IB = 512           # i-block (query block, moving free dim)
NIB = S // IB
NJT = S // 128     # key tiles of 128
GW = 4096          # EG table width (needs >= S + IB)
EPS = 1e-5
MASKED_THRESH = -1e8   # additive mask values below this mean "fully masked"

TRACE = bool(int(os.environ.get("KBA_TRACE", "0")))

LAG = 6  # scores->o-matmul emission lag (bounds live probs tiles)

LAST_RUN_INFO = {}

MM_DTYPE = "bf16"  # kept for test.py printout


# ---------------------------------------------------------------- tile patch
def _apply_tile_patch():
    """walrus CoreV3 codegen tolerates only one sync-wait on an InstDrain;
    the tile-exit drain waits on the whole global clock. Spread the waits
    across extra SP nops."""
    import concourse.tile as tile
    from concourse import mybir
    from concourse.vector_clock import ScopedClock

    if getattr(tile.TileContext, "_kba_patched", False):
        return

    def _drain_and_barrier(self, tick_clock, wait_clock):
        nc = self.nc
        drain_inst = nc.sync.drain()
        wait_clock.add_sem_waits(
            drain_inst.ins, ScopedClock({None: tick_clock.global_clock})
        )
        si = drain_inst.ins.sync_info
        waits = list(si.on_wait or [])
        if len(waits) > 1:
            si.on_wait = waits[:1]
            for i in range(1, len(waits)):
                nop = nc.sync.nop(nofuse=True)
                nop.ins.sync_info = mybir.SyncInfo(
                    on_wait=waits[i : i + 1], on_update=[]
                )
        nc.all_engine_barrier()
        assert self.sems is not None
        popped = nc._tile_sem_poison_stack.pop()
        assert popped is self._sem_poison
        nc.clear_and_free_semaphores(list(self.sems.allocated().values()))
        nc.all_engine_barrier()

    tile.TileContext._drain_and_barrier = _drain_and_barrier
    tile.TileContext._kba_patched = True

    try:
        import concourse.tile_utils as tile_utils

        tile_utils.max_sbuf_usage = 208 * 1024
    except Exception:
        pass


# ------------------------------------------------------------- host helpers
def _toeplitz_profile(m2):
    """If mask[i, j] == phi(j - i) for all i,j, return phi (length 2S-1,
    index t + S - 1), else None."""
    phi = np.empty(2 * S - 1, dtype=np.float32)
    phi[S - 1 :] = m2[0, :]
    phi[: S - 1] = m2[1:, 0][::-1]
    idx = (np.arange(S)[None, :] - np.arange(S)[:, None]) + (S - 1)
    if np.array_equal(phi[idx], m2):
        return phi
    return None


def _eg_table(head, shape, scale, loc, start_pos, phi):
    """[128, GW] bf16: EG[p, u] = exp(prior(d) + phi(-d)), where
    d = i - j = u - p - (S - 1). Masked / out-of-range entries are 0."""
    p = np.arange(128, dtype=np.int64)[:, None]
    u = np.arange(GW, dtype=np.int64)[None, :]
    d = u - p - (S - 1)          # i - j
    dist = (-d - start_pos).astype(np.float32)  # k_pos - q_pos
    sh = np.float32(shape[0, head, 0, 0])
    sc = np.float32(scale[0, head, 0, 0])
    lo = np.float32(loc[0, head, 0, 0])
    loc_t = np.float32(np.exp(lo) - np.exp(-lo))
    z = (dist - loc_t) * np.exp(sc, dtype=np.float32)
    g = -np.power(np.abs(z) + np.float32(EPS), sh, dtype=np.float32)
    if phi is not None:
        t = np.clip(-d + (S - 1), 0, 2 * S - 2)
        g = g + phi[t]
        g[(-d < -(S - 1)) | (-d > (S - 1))] = -np.inf  # out of range: exp -> 0
    with np.errstate(over="ignore", under="ignore"):
        eg = np.exp(g, dtype=np.float32)
    return np.ascontiguousarray(eg.astype(ml_dtypes.bfloat16))


def _kept_tiles(m2):
    """kept[ib] = list of key-tile indices jt whose [128 x IB] block is not
    fully masked. Must be identical for every core (single SPMD program)."""
    kept = []
    for ib in range(NIB):
        row = []
        for jt in range(NJT):
            blk = m2[ib * IB : (ib + 1) * IB, jt * 128 : (jt + 1) * 128]
            if blk.max() > MASKED_THRESH:
                row.append(jt)
        kept.append(row)
    return kept


BAND_THR = 30.0  # drop key tiles once the prior bias is below -BAND_THR


def _prior_reach(head, shape, scale, loc):
    """Smallest distance d* such that the prior penalty is <= -BAND_THR for
    every d >= d* (attention weight < e^-30 of an undecayed key). 2*S if the
    prior never decays that far."""
    sh = np.float32(shape[0, head, 0, 0])
    sc = np.float32(scale[0, head, 0, 0])
    lo = np.float32(loc[0, head, 0, 0])
    loc_t = np.exp(lo) - np.exp(-lo)
    d = np.arange(2 * S, dtype=np.float32)
    pen = (np.abs(-d - loc_t) * np.exp(sc) + np.float32(EPS)) ** sh
    ok = pen >= BAND_THR
    if not ok.any():
        return 2 * S
    # first index from which ok holds for all larger d
    rev_ok = np.minimum.accumulate(ok[::-1])[::-1]
    idx = np.argmax(rev_ok)
    if not rev_ok[idx]:
        return 2 * S
    return int(idx)


HEADS_OF_CORE = [[c, c + H // 2] for c in range(N_CORES)]
# Slot s of every core runs the same instruction stream; slot 0 holds the
# steep-slope (short-reach) heads 0..7 so its key-tile band can be cut
# uniformly, slot 1 holds the flat heads 8..15 (full causal reach).


def _banded_kept(kept, shape, scale, loc):
    """Per-slot kept-tile lists: drop tiles whose whole [IB x 128] block is
    beyond every covered head's prior reach. Identical across cores by
    construction (slot reach = max over cores)."""
    kept_slots = []
    for s in range(HL):
        reach = max(
            _prior_reach(HEADS_OF_CORE[c][s], shape, scale, loc)
            for c in range(N_CORES)
        )
        rows = []
        for ib in range(NIB):
            i0 = ib * IB
            rows.append(
                tuple(
                    jt for jt in kept[ib] if jt * 128 + 127 >= i0 - reach
                )
            )
        kept_slots.append(tuple(rows))
    return tuple(kept_slots)


# ------------------------------------------------------------ program build
_PROGRAM_CACHE = {}


def _build_program(kept_key, fold_mask, repeat=1):
    key = (kept_key, fold_mask, repeat)
    if key in _PROGRAM_CACHE:
        return _PROGRAM_CACHE[key]

    import concourse.bass as bass
    import concourse.tile as tile
    from concourse import bacc, mybir
    from concourse.masks import make_identity

    _apply_tile_patch()

    f32 = mybir.dt.float32
    bf16 = mybir.dt.bfloat16

    kept = [[list(row) for row in slot_rows] for slot_rows in kept_key]

    nc = bacc.Bacc(
        "TRN2", target_bir_lowering=False, debug=False, num_devices=N_CORES
    )
    xT_d = nc.dram_tensor("xT", [S, S], bf16, kind="ExternalInput")
    wq_d = nc.dram_tensor("wq", [S, DL], bf16, kind="ExternalInput")
    wk_d = nc.dram_tensor("wk", [S, DL], bf16, kind="ExternalInput")
    wv_d = nc.dram_tensor("wv", [S, DL], bf16, kind="ExternalInput")
    wo_d = nc.dram_tensor("wo", [DL, S], bf16, kind="ExternalInput")
    g_d = nc.dram_tensor("g", [HL, 128, GW], bf16, kind="ExternalInput")
    if not fold_mask:
        maskT_d = nc.dram_tensor("maskT", [S, S], f32, kind="ExternalInput")
    outT_d = nc.dram_tensor("outT", [S, S], bf16, kind="ExternalOutput")

    Exp = mybir.ActivationFunctionType.Exp
    Copy = mybir.ActivationFunctionType.Copy

    with tile.TileContext(nc) as tc:
        import contextlib

        with contextlib.ExitStack() as ctx:
            consts = ctx.enter_context(tc.tile_pool(name="consts", bufs=1))
            persist = ctx.enter_context(tc.tile_pool(name="persist", bufs=1))
            xpool = ctx.enter_context(tc.tile_pool(name="xp", bufs=2))
            qpool = ctx.enter_context(tc.tile_pool(name="qp", bufs=2))
            vtpool = ctx.enter_context(tc.tile_pool(name="vt", bufs=2))
            pbpool = ctx.enter_context(tc.tile_pool(name="pb", bufs=3))
            prpool = ctx.enter_context(tc.tile_pool(name="pr", bufs=LAG + 2))
            rapool = ctx.enter_context(tc.tile_pool(name="ra", bufs=4))
            rpool = ctx.enter_context(tc.tile_pool(name="rp", bufs=2))
            ospool = ctx.enter_context(tc.tile_pool(name="os", bufs=2))
            obpool = ctx.enter_context(tc.tile_pool(name="ob", bufs=2))
            mpool = (
                ctx.enter_context(tc.tile_pool(name="mk", bufs=3))
                if not fold_mask
                else None
            )
            # PSUM: "big" (6 banks) round-robins phase-1 projection
            # accumulators (4+2), phase-2 scores tiles, and phase-3 output
            # tiles. "acc" (2 banks) serves v-transposes (phase 1) and the
            # o/softmax-sum accumulators (phase 2).
            big = ctx.enter_context(tc.tile_pool(name="big", bufs=6, space="PSUM"))
            acc = ctx.enter_context(tc.tile_pool(name="acc", bufs=2, space="PSUM"))

            # ---- constants / weights (batched DMAs, bf16 direct) ----
            # Split w into chunk-halves ordered by first use so the first
            # projection matmuls start ~4us earlier.
            wq_sb = consts.tile([128, NJT, DL], bf16, name="wq")
            wk_sb = consts.tile([128, NJT, DL], bf16, name="wk")
            wv_sb = consts.tile([128, NJT, DL], bf16, name="wv")
            wo_sb = consts.tile([128, HL, S], bf16, name="wo")
            g_sb = consts.tile([128, HL, GW], bf16, name="g")
            qc = NJT // 4
            for c0, c1 in ((0, qc), (qc, NJT // 2), (NJT // 2, NJT)):
                for w_sb, w_d in ((wq_sb, wq_d), (wk_sb, wk_d)):
                    nc.scalar.dma_start(
                        out=w_sb[:, c0:c1, :],
                        in_=w_d.ap()[c0 * 128 : c1 * 128, :].rearrange(
                            "(c p) d -> p c d", p=128
                        ),
                    )
            nc.scalar.dma_start(
                out=wv_sb[:], in_=wv_d.ap().rearrange("(c p) d -> p c d", p=128)
            )
            nc.scalar.dma_start(
                out=wo_sb[:], in_=wo_d.ap().rearrange("(h p) e -> p h e", p=128)
            )
            nc.scalar.dma_start(
                out=g_sb[:], in_=g_d.ap().rearrange("h p u -> p h u")
            )
            # [128, 128] all-ones stationary: the softmax-sum matmul then
            # produces Sum broadcast across all 128 partitions at no extra
            # PE cost (cycles scale with the moving width, not stationary m).
            ones_sb = consts.tile([128, 128], bf16, name="ones_sb")
            nc.vector.memset(ones_sb[:], 1.0)
            ident = consts.tile([128, 128], bf16)
            make_identity(nc, ident[:])


            kT = persist.tile([128, HL, S], bf16)          # [d, h, j]
            v_sb = persist.tile([128, HL, NJT, HD], bf16)  # [j, h, jt, d]

            qTb_of = {}
            o_sb_of = {}
            xt_of = {}

            def emit_xt_dma(ib, first=False):
                """Prefetch x^T for i-block ib (issued a pipeline step ahead
                of its pass-A consumers so the transfer is fully hidden).
                The prologue block streams in quarters so the very first
                projection matmuls can start ~4us earlier."""
                isl = bass.ts(ib, IB)
                xt = xpool.tile([128, NJT, IB], bf16, name="xt")
                xt_of[ib] = xt
                step = NJT // 4 if first else NJT // 2
                for c0 in range(0, NJT, step):
                    nc.sync.dma_start(
                        out=xt[:, c0 : c0 + step, :],
                        in_=xT_d.ap()[c0 * 128 : (c0 + step) * 128, isl].rearrange(
                            "(c p) i -> p c i", p=128
                        ),
                    )

            def phase1_passA_units(ib):
                """Generator of pass-A work units for i-block ib: q^T/k^T
                projection matmuls per contraction chunk, then the qTb/kT
                copies. Yield granularity ~1.3us of PE work so units can be
                interleaved into phase 2 as PE filler."""
                isl = bass.ts(ib, IB)
                xt = xt_of[ib]
                qk_ps = {}
                for proj in range(2):
                    for dt_i in range(HL):
                        qk_ps[(proj, dt_i)] = big.tile(
                            [128, IB], f32, tag="big", name=f"ps{proj}{dt_i}"
                        )
                for ct in range(NJT):
                    for proj, w_sb in enumerate((wq_sb, wk_sb)):
                        for dt_i in range(HL):
                            nc.tensor.matmul(
                                qk_ps[(proj, dt_i)][:],
                                lhsT=w_sb[:, ct, dt_i * HD : (dt_i + 1) * HD],
                                rhs=xt[:, ct, :],
                                start=(ct == 0),
                                stop=(ct == NJT - 1),
                            )
                    yield
                qTb = qpool.tile([128, HL, IB], bf16, name="qTb")
                qTb_of[ib] = qTb
                for dt_i in range(HL):
                    nc.scalar.activation(qTb[:, dt_i, :], qk_ps[(0, dt_i)][:], Copy)
                    nc.vector.tensor_copy(kT[:, dt_i, isl], qk_ps[(1, dt_i)][:])
                yield

            def emit_phase1_passB(ib):
                """v^T projection + v transpose into v_sb for i-block ib.
                Emitted between phase2(prev) and phase3(prev) to fill the
                softmax-tail join with independent PE work."""
                xt = xt_of.pop(ib)
                v_ps = [
                    big.tile([128, IB], f32, tag="big", name=f"vps{dt_i}")
                    for dt_i in range(HL)
                ]
                for ct in range(NJT):
                    for dt_i in range(HL):
                        nc.tensor.matmul(
                            v_ps[dt_i][:],
                            lhsT=wv_sb[:, ct, dt_i * HD : (dt_i + 1) * HD],
                            rhs=xt[:, ct, :],
                            start=(ct == 0),
                            stop=(ct == NJT - 1),
                        )
                for dt_i in range(HL):
                    vt = vtpool.tile([128, IB], bf16, name="vt")
                    nc.scalar.activation(vt[:], v_ps[dt_i][:], Copy)
                    # transpose v^T [d, j] -> v [j, d] in 128-blocks
                    for s4 in range(IB // 128):
                        jt = (ib * IB) // 128 + s4
                        tp = acc.tile([128, 128], bf16, tag="acc", name="tp")
                        nc.tensor.transpose(
                            tp[:], vt[:, s4 * 128 : (s4 + 1) * 128], ident[:]
                        )
                        nc.vector.tensor_copy(v_sb[:, dt_i, jt, :], tp[:])

            def emit_phase2(ib, filler=None):
                """Attention for both local heads of i-block ib. `filler` is
                an optional iterator of next-block pass-A units, drained
                evenly across the score tiles to give PE independent work
                while the exp/multiply chain catches up."""
                isl = bass.ts(ib, IB)
                qTb = qTb_of[ib]
                o_sb = ospool.tile([128, HL, IB], bf16, name="o_sb")
                o_sb_of[ib] = o_sb
                n_tiles = sum(len(kept[h][ib]) for h in range(HL))
                tiles_done = 0
                filler_done = 0
                filler_total = NJT + 1  # passA yields per ct chunk + copies

                def drain_filler():
                    nonlocal filler_done
                    if filler is None:
                        return
                    while filler_done / filler_total <= tiles_done / n_tiles:
                        try:
                            next(filler)
                            filler_done += 1
                        except StopIteration:
                            break
                for h in range(HL):
                    jts = kept[h][ib]
                    # Causal narrowing: a diagonal key tile jt only affects
                    # queries i >= jt*128, i.e. moving columns [off:], with
                    # off = jt*128 - ib*IB. Skipped columns stay correct: the
                    # first tile (off 0) writes the accumulators full-width,
                    # and no later tile contributes below its own off.
                    offs = [max(0, jt * 128 - ib * IB) for jt in jts]
                    if offs[0] != 0 or not fold_mask:
                        offs = [0] * len(jts)
                    oacc = acc.tile([128, IB], f32, tag="acc", name="oacc")
                    probs = {}
                    # Softmax denominator: instead of one ones-matmul per key
                    # tile (PE rows), accumulate the probability tiles on DVE
                    # in two interleaved bf16 chains (2x perf mode) and do a
                    # single partition-sum matmul per head-block at the end.
                    racc = [None, None]

                    def emit_pv(idx):
                        jt = jts[idx]
                        off = offs[idx]
                        nc.tensor.matmul(
                            oacc[:, off:],
                            lhsT=v_sb[:, h, jt, :],
                            rhs=probs[idx][:, off:],
                            start=(idx == 0),
                            stop=(idx == len(jts) - 1),
                        )

                    for idx, jt in enumerate(jts):
                        drain_filler()
                        tiles_done += 1
                        off = offs[idx]
                        sc = big.tile([128, IB], f32, tag="big", name="sc")
                        nc.tensor.matmul(
                            sc[:, off:],
                            lhsT=kT[:, h, jt * 128 : (jt + 1) * 128],
                            rhs=qTb[:, h, off:],
                            start=True,
                            stop=True,
                        )
                        if not fold_mask:
                            mt = mpool.tile([128, IB], f32)
                            nc.sync.dma_start(
                                out=mt[:],
                                in_=maskT_d[jt * 128 : (jt + 1) * 128, isl],
                            )
                            nc.vector.tensor_add(
                                sc[:, off:], sc[:, off:], mt[:, off:]
                            )
                        pb = pbpool.tile([128, IB], bf16, name="pb")
                        nc.scalar.activation(pb[:, off:], sc[:, off:], Exp)
                        base = ib * IB - jt * 128 + (S - 1)
                        pr = prpool.tile([128, IB], bf16, name="pr")
                        if off > 0:
                            # causally-invalid columns must read as 0 in the
                            # denominator chains
                            nc.vector.memset(pr[:, :off], 0.0)
                        nc.vector.tensor_mul(
                            pr[:, off:],
                            pb[:, off:],
                            g_sb[:, h, base + off : base + IB],
                        )
                        probs[idx] = pr
                        ch = idx % 2
                        if racc[ch] is None:
                            racc[ch] = rapool.tile(
                                [128, IB], bf16, name=f"racc{ch}"
                            )
                            nc.vector.tensor_copy(racc[ch][:], pr[:])
                        else:
                            nc.vector.tensor_add(
                                racc[ch][:], racc[ch][:], pr[:]
                            )
                        if idx - LAG >= 0:
                            emit_pv(idx - LAG)
                    for idx in range(max(0, len(jts) - LAG), len(jts)):
                        emit_pv(idx)

                    if racc[1] is not None:
                        nc.vector.tensor_add(racc[0][:], racc[0][:], racc[1][:])
                    sacc = acc.tile([128, IB], f32, tag="acc", name="sacc")
                    nc.tensor.matmul(
                        sacc[:],
                        lhsT=ones_sb[:],
                        rhs=racc[0][:],
                        start=True,
                        stop=True,
                    )
                    rbc = rpool.tile([128, IB], f32, tag="rbc", name="rbc")
                    nc.vector.reciprocal(rbc[:], sacc[:])
                    nc.vector.tensor_mul(o_sb[:, h, :], oacc[:], rbc[:])
                if filler is not None:
                    for _ in filler:
                        pass

            def emit_phase3(ib):
                """out^T = wo^T o^T for i-block ib (partial; host sums cores).
                Output DMA goes out in two halves so the transfer overlaps the
                remaining copies."""
                isl = bass.ts(ib, IB)
                o_sb = o_sb_of.pop(ib)
                qTb_of.pop(ib, None)
                ob = obpool.tile([128, NJT, IB], bf16, name="ob")
                halves = (range(0, NJT // 2), range(NJT // 2, NJT))
                for hi, ets in enumerate(halves):
                    for et in ets:
                        po = big.tile([128, IB], f32, tag="big", name="po")
                        for h in range(HL):
                            nc.tensor.matmul(
                                po[:],
                                lhsT=wo_sb[:, h, et * 128 : (et + 1) * 128],
                                rhs=o_sb[:, h, :],
                                start=(h == 0),
                                stop=(h == HL - 1),
                            )
                        if et % 2 == 0:
                            nc.scalar.activation(ob[:, et, :], po[:], Copy)
                        else:
                            nc.vector.tensor_copy(ob[:, et, :], po[:])
                    e0, e1 = ets[0], ets[-1] + 1
                    nc.sync.dma_start(
                        out=outT_d.ap()[e0 * 128 : e1 * 128, isl].rearrange(
                            "(e p) i -> p e i", p=128
                        ),
                        in_=ob[:, e0:e1, :],
                    )

            # Software-pipelined schedule: the next block's pass-A projection
            # units are interleaved INTO attention(ib) as PE filler, its
            # v-projection (pass B) fills the softmax-tail -> output join,
            # and x prefetches run a full step ahead.
            blocks = [ib for _rep in range(repeat) for ib in range(NIB)]
            emit_xt_dma(blocks[0], first=True)
            for _ in phase1_passA_units(blocks[0]):
                pass
            if len(blocks) > 1:
                emit_xt_dma(blocks[1])
            emit_phase1_passB(blocks[0])
            for n, ib in enumerate(blocks):
                nxt = blocks[n + 1] if n + 1 < len(blocks) else None
                emit_phase2(
                    ib,
                    filler=phase1_passA_units(nxt) if nxt is not None else None,
                )
                if nxt is not None:
                    emit_phase1_passB(nxt)
                if n + 2 < len(blocks):
                    emit_xt_dma(blocks[n + 2])
                emit_phase3(ib)

    nc.compile()
    _PROGRAM_CACHE[key] = nc
    return nc


# ------------------------------------------------------------------- kernel
def prepare(x, mask, wq, wk, wv, wo, shape, scale, loc, start_pos):
    """Host prep: build/cache program and per-core input maps."""
    x32 = np.asarray(x, np.float32).reshape(S, DIM)
    m2 = np.asarray(mask, np.float32).reshape(S, S)
    wq32 = np.asarray(wq, np.float32)
    wk32 = np.asarray(wk, np.float32)
    wv32 = np.asarray(wv, np.float32)
    wo32 = np.asarray(wo, np.float32)
    shape = np.asarray(shape, np.float32)
    scale = np.asarray(scale, np.float32)
    loc = np.asarray(loc, np.float32)
    sp = int(start_pos)

    phi = _toeplitz_profile(m2)
    fold_mask = phi is not None
    kept = _kept_tiles(m2)
    kept_key = _banded_kept(kept, shape, scale, loc)

    LAST_RUN_INFO["build_args"] = (kept_key, fold_mask)
    nc = _build_program(kept_key, fold_mask)

    bf = ml_dtypes.bfloat16
    xT = np.ascontiguousarray(x32.T).astype(bf)
    inv_s = np.float32(1.0 / math.sqrt(HD))
    maskT = None if fold_mask else np.ascontiguousarray(m2.T)

    in_maps = []
    for c in range(N_CORES):
        heads = HEADS_OF_CORE[c]
        cols = np.concatenate(
            [np.arange(h * HD, (h + 1) * HD) for h in heads]
        )
        im = {
            "xT": xT,
            "wq": np.ascontiguousarray(wq32[:, cols] * inv_s).astype(bf),
            "wk": np.ascontiguousarray(wk32[:, cols]).astype(bf),
            "wv": np.ascontiguousarray(wv32[:, cols]).astype(bf),
            "wo": np.ascontiguousarray(wo32[cols, :]).astype(bf),
            "g": np.stack(
                [_eg_table(h, shape, scale, loc, sp, phi) for h in heads]
            ),
        }
        if not fold_mask:
            im["maskT"] = maskT
        in_maps.append(im)
    return nc, in_maps


def _reduce(results):
    acc = results[0]["outT"].astype(np.float32)
    for c in range(1, N_CORES):
        acc = acc + results[c]["outT"].astype(np.float32)
    return np.ascontiguousarray(acc.T)[None].astype(np.float32)


_RUNNER_CACHE = {}


def _get_runner(nc):
    """Build (once) a reusable jitted 8-core runner for the program `nc`.
    Mirrors bass2jax.run_bass_via_pjrt's multi-core path without output
    donation (outT is fully written by the kernel) so it can be re-invoked."""
    if id(nc) in _RUNNER_CACHE:
        return _RUNNER_CACHE[id(nc)]

    import jax
    from jax.sharding import Mesh, NamedSharding, PartitionSpec

    from jax.experimental.shard_map import shard_map
    from concourse import mybir
    from concourse.bass2jax import (
        _bass_exec_p,
        install_neuronx_cc_hook,
        partition_id_tensor,
    )

    install_neuronx_cc_hook()
    partition_name = nc.partition_id_tensor.name if nc.partition_id_tensor else None

    in_names, out_names, out_avals = [], [], []
    for alloc in nc.m.functions[0].allocations:
        if not isinstance(alloc, mybir.MemoryLocationSet):
            continue
        name = alloc.memorylocations[0].name
        if alloc.kind == "ExternalInput":
            if name != partition_name:
                in_names.append(name)
        elif alloc.kind == "ExternalOutput":
            out_names.append(name)
            out_avals.append(
                jax.core.ShapedArray(
                    tuple(alloc.tensor_shape), mybir.dt.np(alloc.dtype)
                )
            )
    n_params = len(in_names)
    all_names = in_names + out_names
    if partition_name is not None:
        all_names = all_names + [partition_name]

    def _body(*args):
        operands = list(args)
        if partition_name is not None:
            operands.append(partition_id_tensor())
        return tuple(
            _bass_exec_p.bind(
                *operands,
                out_avals=tuple(out_avals),
                in_names=tuple(all_names),
                out_names=tuple(out_names),
                lowering_input_output_aliases=(),
                sim_require_finite=True,
                sim_require_nnan=True,
                nc=nc,
            )
        )

    devices = jax.devices()[:N_CORES]
    mesh = Mesh(np.asarray(devices), ("core",))
    sharded = jax.jit(
        shard_map(
            _body,
            mesh=mesh,
            in_specs=(PartitionSpec("core"),) * (n_params + len(out_names)),
            out_specs=(PartitionSpec("core"),) * len(out_names),
            check_rep=False,
        ),
        keep_unused=True,
    )
    sh = NamedSharding(mesh, PartitionSpec("core"))

    def run(in_maps):
        concat_in = [
            np.concatenate(
                [np.asarray(in_maps[c][nm]) for c in range(N_CORES)], axis=0
            )
            for nm in in_names
        ]
        concat_zero = [
            np.zeros((N_CORES * av.shape[0], *av.shape[1:]), av.dtype)
            for av in out_avals
        ]
        dev_args = [jax.device_put(a, sh) for a in concat_in + concat_zero]
        out_arrs = sharded(*dev_args)
        return [
            {
                nm: np.asarray(out_arrs[i]).reshape(
                    N_CORES, *out_avals[i].shape
                )[c]
                for i, nm in enumerate(out_names)
            }
            for c in range(N_CORES)
        ]

    _RUNNER_CACHE[id(nc)] = run
    return run


def kernel(x, mask, wq, wk, wv, wo, shape, scale, loc, start_pos):
    nc, in_maps = prepare(x, mask, wq, wk, wv, wo, shape, scale, loc, start_pos)

    if os.environ.get("KBA_SIM", "0") == "1":
        from concourse import bass_interp

        n_sim = int(os.environ.get("KBA_SIM_CORES", str(N_CORES)))
        sim = bass_interp.MultiCoreSim(nc, n_sim)
        for c in range(n_sim):
            for k, v in in_maps[c].items():
                sim.cores[c].tensor(k)[:] = v
        sim.simulate()
        results = [
            {"outT": np.array(sim.cores[c].tensor("outT"))}
            for c in range(n_sim)
        ] + [
            {"outT": np.zeros((S, S), ml_dtypes.bfloat16)}
            for _ in range(N_CORES - n_sim)
        ]
        LAST_RUN_INFO["exec_time_ns"] = None
    else:
        results = _get_runner(nc)(in_maps)
        LAST_RUN_INFO["exec_time_ns"] = None

    LAST_RUN_INFO["results"] = results
    return _reduce(results)


# revision 37
# speedup vs baseline: 2.1480x; 1.1304x over previous
"""BayesianAttention (power-law prior + causal mask) on 8 trn2 cores.

Self-contained: builds a Bass/Tile kernel, shards heads across 8 NeuronCores
(2 heads per core; wq/wk/wv column-sharded, wo row-sharded), runs via a
jitted 8-core PJRT runner, and reduces the partial outputs on host.

Device-side layout is fully transposed (contraction dims on partitions) and
all matmul operands are bf16 (f32 PSUM accumulation):
  host sends x^T [c, i] bf16; device computes q^T/k^T/v^T = W^T x^T,
  transposes v, s^T[j,i] = k^T_j . q^T_i (raw, no bias),
  probs = exp(s^T) * EG, where EG = exp(prior + causal mask) is a
  precomputed per-head Toeplitz table ([128, 4096] bf16, masked entries
  exactly 0), o^T[d,i] = v^T probs / (w^T probs), out^T[e,i] = wo^T o^T.
Host returns sum_c(out^T_c)^T.

The kernel is emitted as one software-pipelined loop over 512-wide query
blocks: projections(ib) -> attention(ib, both heads) -> output(ib), which
keeps the tensor engine continuously busy (PSUM pool backpressure paces the
phases); x / out move as a few large batched DMAs on the two HWDGE queues.
"""

import math
import os

import ml_dtypes
import numpy as np

S = 2048          # sequence length
DIM = 2048        # model dim
H = 16            # heads
HD = 128          # head dim
N_CORES = 8
HL = H // N_CORES  # heads per core (2)
DL = HL * HD       # local projected dim (256)
IB = 512           # i-block (query block, moving free dim)
NIB = S // IB
NJT = S // 128     # key tiles of 128
GW = 4096          # EG table width (needs >= S + IB)
EPS = 1e-5
MASKED_THRESH = -1e8   # additive mask values below this mean "fully masked"

TRACE = bool(int(os.environ.get("KBA_TRACE", "0")))

LAG = 6  # scores->o-matmul emission lag (bounds live probs tiles)

LAST_RUN_INFO = {}

MM_DTYPE = "bf16"  # kept for test.py printout


# ---------------------------------------------------------------- tile patch
def _apply_tile_patch():
    """walrus CoreV3 codegen tolerates only one sync-wait on an InstDrain;
    the tile-exit drain waits on the whole global clock. Spread the waits
    across extra SP nops."""
    import concourse.tile as tile
    from concourse import mybir
    from concourse.vector_clock import ScopedClock

    if getattr(tile.TileContext, "_kba_patched", False):
        return

    def _drain_and_barrier(self, tick_clock, wait_clock):
        nc = self.nc
        drain_inst = nc.sync.drain()
        wait_clock.add_sem_waits(
            drain_inst.ins, ScopedClock({None: tick_clock.global_clock})
        )
        si = drain_inst.ins.sync_info
        waits = list(si.on_wait or [])
        if len(waits) > 1:
            si.on_wait = waits[:1]
            for i in range(1, len(waits)):
                nop = nc.sync.nop(nofuse=True)
                nop.ins.sync_info = mybir.SyncInfo(
                    on_wait=waits[i : i + 1], on_update=[]
                )
        nc.all_engine_barrier()
        assert self.sems is not None
        popped = nc._tile_sem_poison_stack.pop()
        assert popped is self._sem_poison
        nc.clear_and_free_semaphores(list(self.sems.allocated().values()))
        nc.all_engine_barrier()

    tile.TileContext._drain_and_barrier = _drain_and_barrier
    tile.TileContext._kba_patched = True

    try:
        import concourse.tile_utils as tile_utils

        tile_utils.max_sbuf_usage = 208 * 1024
    except Exception:
        pass


# ------------------------------------------------------------- host helpers
def _toeplitz_profile(m2):
    """If mask[i, j] == phi(j - i) for all i,j, return phi (length 2S-1,
    index t + S - 1), else None."""
    phi = np.empty(2 * S - 1, dtype=np.float32)
    phi[S - 1 :] = m2[0, :]
    phi[: S - 1] = m2[1:, 0][::-1]
    idx = (np.arange(S)[None, :] - np.arange(S)[:, None]) + (S - 1)
    if np.array_equal(phi[idx], m2):
        return phi
    return None


def _eg_table(head, shape, scale, loc, start_pos, phi):
    """[128, GW] bf16: EG[p, u] = exp(prior(d) + phi(-d)), where
    d = i - j = u - p - (S - 1). Masked / out-of-range entries are 0."""
    p = np.arange(128, dtype=np.int64)[:, None]
    u = np.arange(GW, dtype=np.int64)[None, :]
    d = u - p - (S - 1)          # i - j
    dist = (-d - start_pos).astype(np.float32)  # k_pos - q_pos
    sh = np.float32(shape[0, head, 0, 0])
    sc = np.float32(scale[0, head, 0, 0])
    lo = np.float32(loc[0, head, 0, 0])
    loc_t = np.float32(np.exp(lo) - np.exp(-lo))
    z = (dist - loc_t) * np.exp(sc, dtype=np.float32)
    g = -np.power(np.abs(z) + np.float32(EPS), sh, dtype=np.float32)
    if phi is not None:
        t = np.clip(-d + (S - 1), 0, 2 * S - 2)
        g = g + phi[t]
        g[(-d < -(S - 1)) | (-d > (S - 1))] = -np.inf  # out of range: exp -> 0
    with np.errstate(over="ignore", under="ignore"):
        eg = np.exp(g, dtype=np.float32)
    return np.ascontiguousarray(eg.astype(ml_dtypes.bfloat16))


def _kept_tiles(m2):
    """kept[ib] = list of key-tile indices jt whose [128 x IB] block is not
    fully masked. Must be identical for every core (single SPMD program)."""
    kept = []
    for ib in range(NIB):
        row = []
        for jt in range(NJT):
            blk = m2[ib * IB : (ib + 1) * IB, jt * 128 : (jt + 1) * 128]
            if blk.max() > MASKED_THRESH:
                row.append(jt)
        kept.append(row)
    return kept


BAND_THR = 30.0  # drop key tiles once the prior bias is below -BAND_THR


def _prior_reach(head, shape, scale, loc):
    """Smallest distance d* such that the prior penalty is <= -BAND_THR for
    every d >= d* (attention weight < e^-30 of an undecayed key). 2*S if the
    prior never decays that far."""
    sh = np.float32(shape[0, head, 0, 0])
    sc = np.float32(scale[0, head, 0, 0])
    lo = np.float32(loc[0, head, 0, 0])
    loc_t = np.exp(lo) - np.exp(-lo)
    d = np.arange(2 * S, dtype=np.float32)
    pen = (np.abs(-d - loc_t) * np.exp(sc) + np.float32(EPS)) ** sh
    ok = pen >= BAND_THR
    if not ok.any():
        return 2 * S
    # first index from which ok holds for all larger d
    rev_ok = np.minimum.accumulate(ok[::-1])[::-1]
    idx = np.argmax(rev_ok)
    if not rev_ok[idx]:
        return 2 * S
    return int(idx)


HEADS_OF_CORE = [[c, c + H // 2] for c in range(N_CORES)]
# Slot s of every core runs the same instruction stream; slot 0 holds the
# steep-slope (short-reach) heads 0..7 so its key-tile band can be cut
# uniformly, slot 1 holds the flat heads 8..15 (full causal reach).


def _banded_kept(kept, shape, scale, loc):
    """Per-slot kept-tile lists: drop tiles whose whole [IB x 128] block is
    beyond every covered head's prior reach. Identical across cores by
    construction (slot reach = max over cores)."""
    kept_slots = []
    for s in range(HL):
        reach = max(
            _prior_reach(HEADS_OF_CORE[c][s], shape, scale, loc)
            for c in range(N_CORES)
        )
        rows = []
        for ib in range(NIB):
            i0 = ib * IB
            rows.append(
                tuple(
                    jt for jt in kept[ib] if jt * 128 + 127 >= i0 - reach
                )
            )
        kept_slots.append(tuple(rows))
    return tuple(kept_slots)


# ------------------------------------------------------------ program build
_PROGRAM_CACHE = {}


def _build_program(kept_key, fold_mask, repeat=1):
    key = (kept_key, fold_mask, repeat)
    if key in _PROGRAM_CACHE:
        return _PROGRAM_CACHE[key]

    import concourse.bass as bass
    import concourse.tile as tile
    from concourse import bacc, mybir
    from concourse.masks import make_identity

    _apply_tile_patch()

    f32 = mybir.dt.float32
    bf16 = mybir.dt.bfloat16

    kept = [[list(row) for row in slot_rows] for slot_rows in kept_key]

    nc = bacc.Bacc(
        "TRN2", target_bir_lowering=False, debug=False, num_devices=N_CORES
    )
    xT_d = nc.dram_tensor("xT", [S, S], bf16, kind="ExternalInput")
    wq_d = nc.dram_tensor("wq", [S, DL], bf16, kind="ExternalInput")
    wk_d = nc.dram_tensor("wk", [S, DL], bf16, kind="ExternalInput")
    wv_d = nc.dram_tensor("wv", [S, DL], bf16, kind="ExternalInput")
    wo_d = nc.dram_tensor("wo", [DL, S], bf16, kind="ExternalInput")
    g_d = nc.dram_tensor("g", [HL, 128, GW], bf16, kind="ExternalInput")
    if not fold_mask:
        maskT_d = nc.dram_tensor("maskT", [S, S], f32, kind="ExternalInput")
    outT_d = nc.dram_tensor("outT", [S, S], bf16, kind="ExternalOutput")

    Exp = mybir.ActivationFunctionType.Exp
    Copy = mybir.ActivationFunctionType.Copy

    with tile.TileContext(nc) as tc:
        import contextlib

        with contextlib.ExitStack() as ctx:
            consts = ctx.enter_context(tc.tile_pool(name="consts", bufs=1))
            persist = ctx.enter_context(tc.tile_pool(name="persist", bufs=1))
            xpool = ctx.enter_context(tc.tile_pool(name="xp", bufs=2))
            qpool = ctx.enter_context(tc.tile_pool(name="qp", bufs=2))
            vtpool = ctx.enter_context(tc.tile_pool(name="vt", bufs=2))
            pbpool = ctx.enter_context(tc.tile_pool(name="pb", bufs=3))
            prpool = ctx.enter_context(tc.tile_pool(name="pr", bufs=LAG + 2))
            rapool = ctx.enter_context(tc.tile_pool(name="ra", bufs=4))
            rpool = ctx.enter_context(tc.tile_pool(name="rp", bufs=2))
            ospool = ctx.enter_context(tc.tile_pool(name="os", bufs=2))
            obpool = ctx.enter_context(tc.tile_pool(name="ob", bufs=2))
            mpool = (
                ctx.enter_context(tc.tile_pool(name="mk", bufs=3))
                if not fold_mask
                else None
            )
            # PSUM: "big" (6 banks) round-robins phase-1 projection
            # accumulators (4+2), phase-2 scores tiles, and phase-3 output
            # tiles. "acc" (2 banks) serves v-transposes (phase 1) and the
            # o/softmax-sum accumulators (phase 2).
            big = ctx.enter_context(tc.tile_pool(name="big", bufs=6, space="PSUM"))
            acc = ctx.enter_context(tc.tile_pool(name="acc", bufs=2, space="PSUM"))

            # ---- constants / weights (batched DMAs, bf16 direct) ----
            # Split w into chunk-halves ordered by first use so the first
            # projection matmuls start ~4us earlier.
            wq_sb = consts.tile([128, NJT, DL], bf16, name="wq")
            wk_sb = consts.tile([128, NJT, DL], bf16, name="wk")
            wv_sb = consts.tile([128, NJT, DL], bf16, name="wv")
            wo_sb = consts.tile([128, HL, S], bf16, name="wo")
            g_sb = consts.tile([128, HL, GW], bf16, name="g")
            qc = NJT // 4
            for c0, c1 in ((0, qc), (qc, NJT // 2), (NJT // 2, NJT)):
                for w_sb, w_d in ((wq_sb, wq_d), (wk_sb, wk_d)):
                    nc.scalar.dma_start(
                        out=w_sb[:, c0:c1, :],
                        in_=w_d.ap()[c0 * 128 : c1 * 128, :].rearrange(
                            "(c p) d -> p c d", p=128
                        ),
                    )
            nc.scalar.dma_start(
                out=wv_sb[:], in_=wv_d.ap().rearrange("(c p) d -> p c d", p=128)
            )
            nc.scalar.dma_start(
                out=wo_sb[:], in_=wo_d.ap().rearrange("(h p) e -> p h e", p=128)
            )
            nc.scalar.dma_start(
                out=g_sb[:], in_=g_d.ap().rearrange("h p u -> p h u")
            )
            # [128, 128] all-ones stationary: the softmax-sum matmul then
            # produces Sum broadcast across all 128 partitions at no extra
            # PE cost (cycles scale with the moving width, not stationary m).
            ones_sb = consts.tile([128, 128], bf16, name="ones_sb")
            nc.vector.memset(ones_sb[:], 1.0)
            ident = consts.tile([128, 128], bf16)
            make_identity(nc, ident[:])


            kT = persist.tile([128, HL, S], bf16)          # [d, h, j]
            v_sb = persist.tile([128, HL, NJT, HD], bf16)  # [j, h, jt, d]

            qTb_of = {}
            o_sb_of = {}
            xt_of = {}

            def emit_xt_dma(ib, first=False):
                """Prefetch x^T for i-block ib (issued a pipeline step ahead
                of its pass-A consumers so the transfer is fully hidden).
                The prologue block streams in quarters so the very first
                projection matmuls can start ~4us earlier."""
                isl = bass.ts(ib, IB)
                xt = xpool.tile([128, NJT, IB], bf16, name="xt")
                xt_of[ib] = xt
                step = NJT // 4 if first else NJT // 2
                for c0 in range(0, NJT, step):
                    nc.sync.dma_start(
                        out=xt[:, c0 : c0 + step, :],
                        in_=xT_d.ap()[c0 * 128 : (c0 + step) * 128, isl].rearrange(
                            "(c p) i -> p c i", p=128
                        ),
                    )

            def phase1_passA_units(ib):
                """Generator of pass-A work units for i-block ib: q^T/k^T
                projection matmuls per contraction chunk, then the qTb/kT
                copies. Yield granularity ~1.3us of PE work so units can be
                interleaved into phase 2 as PE filler."""
                isl = bass.ts(ib, IB)
                xt = xt_of[ib]
                qk_ps = {}
                for proj in range(2):
                    for dt_i in range(HL):
                        qk_ps[(proj, dt_i)] = big.tile(
                            [128, IB], f32, tag="big", name=f"ps{proj}{dt_i}"
                        )
                for ct in range(NJT):
                    for proj, w_sb in enumerate((wq_sb, wk_sb)):
                        for dt_i in range(HL):
                            nc.tensor.matmul(
                                qk_ps[(proj, dt_i)][:],
                                lhsT=w_sb[:, ct, dt_i * HD : (dt_i + 1) * HD],
                                rhs=xt[:, ct, :],
                                start=(ct == 0),
                                stop=(ct == NJT - 1),
                            )
                    yield
                qTb = qpool.tile([128, HL, IB], bf16, name="qTb")
                qTb_of[ib] = qTb
                for dt_i in range(HL):
                    nc.scalar.activation(qTb[:, dt_i, :], qk_ps[(0, dt_i)][:], Copy)
                    nc.vector.tensor_copy(kT[:, dt_i, isl], qk_ps[(1, dt_i)][:])
                yield

            def emit_phase1_passB(ib):
                """v^T projection + v transpose into v_sb for i-block ib.
                Emitted between phase2(prev) and phase3(prev) to fill the
                softmax-tail join with independent PE work."""
                xt = xt_of.pop(ib)
                v_ps = [
                    big.tile([128, IB], f32, tag="big", name=f"vps{dt_i}")
                    for dt_i in range(HL)
                ]
                for ct in range(NJT):
                    for dt_i in range(HL):
                        nc.tensor.matmul(
                            v_ps[dt_i][:],
                            lhsT=wv_sb[:, ct, dt_i * HD : (dt_i + 1) * HD],
                            rhs=xt[:, ct, :],
                            start=(ct == 0),
                            stop=(ct == NJT - 1),
                        )
                for dt_i in range(HL):
                    vt = vtpool.tile([128, IB], bf16, name="vt")
                    nc.scalar.activation(vt[:], v_ps[dt_i][:], Copy)
                    # transpose v^T [d, j] -> v [j, d] in 128-blocks
                    for s4 in range(IB // 128):
                        jt = (ib * IB) // 128 + s4
                        tp = acc.tile([128, 128], bf16, tag="acc", name="tp")
                        nc.tensor.transpose(
                            tp[:], vt[:, s4 * 128 : (s4 + 1) * 128], ident[:]
                        )
                        nc.vector.tensor_copy(v_sb[:, dt_i, jt, :], tp[:])

            def emit_phase2(ib, filler=None, ftotal=NJT + 1):
                """Attention for both local heads of i-block ib. `filler` is
                an optional iterator of work units (next-block pass-A, or the
                deferred previous phase 3 for the final block), drained evenly
                across the score tiles to give PE independent work while the
                exp/multiply chain catches up."""
                isl = bass.ts(ib, IB)
                qTb = qTb_of[ib]
                o_sb = ospool.tile([128, HL, IB], bf16, name="o_sb")
                o_sb_of[ib] = o_sb
                n_tiles = sum(len(kept[h][ib]) for h in range(HL))
                tiles_done = 0
                filler_done = 0
                filler_total = ftotal

                def drain_filler():
                    nonlocal filler_done
                    if filler is None:
                        return
                    while filler_done / filler_total <= tiles_done / n_tiles:
                        try:
                            next(filler)
                            filler_done += 1
                        except StopIteration:
                            break
                for h in range(HL):
                    jts = kept[h][ib]
                    # Causal narrowing: a diagonal key tile jt only affects
                    # queries i >= jt*128, i.e. moving columns [off:], with
                    # off = jt*128 - ib*IB. Skipped columns stay correct: the
                    # first tile (off 0) writes the accumulators full-width,
                    # and no later tile contributes below its own off.
                    offs = [max(0, jt * 128 - ib * IB) for jt in jts]
                    if offs[0] != 0 or not fold_mask:
                        offs = [0] * len(jts)
                    oacc = acc.tile([128, IB], f32, tag="acc", name="oacc")
                    probs = {}
                    # Softmax denominator: instead of one ones-matmul per key
                    # tile (PE rows), accumulate the probability tiles on DVE
                    # in two interleaved bf16 chains (2x perf mode) and do a
                    # single partition-sum matmul per head-block at the end.
                    racc = [None, None]

                    def emit_pv(idx):
                        jt = jts[idx]
                        off = offs[idx]
                        nc.tensor.matmul(
                            oacc[:, off:],
                            lhsT=v_sb[:, h, jt, :],
                            rhs=probs[idx][:, off:],
                            start=(idx == 0),
                            stop=(idx == len(jts) - 1),
                        )

                    for idx, jt in enumerate(jts):
                        drain_filler()
                        tiles_done += 1
                        off = offs[idx]
                        sc = big.tile([128, IB], f32, tag="big", name="sc")
                        nc.tensor.matmul(
                            sc[:, off:],
                            lhsT=kT[:, h, jt * 128 : (jt + 1) * 128],
                            rhs=qTb[:, h, off:],
                            start=True,
                            stop=True,
                        )
                        if not fold_mask:
                            mt = mpool.tile([128, IB], f32)
                            nc.sync.dma_start(
                                out=mt[:],
                                in_=maskT_d[jt * 128 : (jt + 1) * 128, isl],
                            )
                            nc.vector.tensor_add(
                                sc[:, off:], sc[:, off:], mt[:, off:]
                            )
                        pb = pbpool.tile([128, IB], bf16, name="pb")
                        nc.scalar.activation(pb[:, off:], sc[:, off:], Exp)
                        base = ib * IB - jt * 128 + (S - 1)
                        pr = prpool.tile([128, IB], bf16, name="pr")
                        if off > 0:
                            # causally-invalid columns must read as 0 in the
                            # denominator chains
                            nc.vector.memset(pr[:, :off], 0.0)
                        nc.vector.tensor_mul(
                            pr[:, off:],
                            pb[:, off:],
                            g_sb[:, h, base + off : base + IB],
                        )
                        probs[idx] = pr
                        ch = idx % 2
                        if racc[ch] is None:
                            racc[ch] = rapool.tile(
                                [128, IB], bf16, name=f"racc{ch}"
                            )
                            nc.vector.tensor_copy(racc[ch][:], pr[:])
                        else:
                            nc.vector.tensor_add(
                                racc[ch][:], racc[ch][:], pr[:]
                            )
                        if idx - LAG >= 0:
                            emit_pv(idx - LAG)
                    for idx in range(max(0, len(jts) - LAG), len(jts)):
                        emit_pv(idx)

                    if racc[1] is not None:
                        nc.vector.tensor_add(racc[0][:], racc[0][:], racc[1][:])
                    sacc = acc.tile([128, IB], f32, tag="acc", name="sacc")
                    nc.tensor.matmul(
                        sacc[:],
                        lhsT=ones_sb[:],
                        rhs=racc[0][:],
                        start=True,
                        stop=True,
                    )
                    rbc = rpool.tile([128, IB], f32, tag="rbc", name="rbc")
                    nc.vector.reciprocal(rbc[:], sacc[:])
                    nc.vector.tensor_mul(o_sb[:, h, :], oacc[:], rbc[:])
                if filler is not None:
                    for _ in filler:
                        pass

            def phase3_units(ib, chunks=2):
                """Generator of per-et output units for i-block ib:
                out^T = wo^T o^T (partial; host sums cores). The output DMA
                goes out in `chunks` pieces so the transfer overlaps the
                remaining copies (quarters for the final block to shorten the
                kernel tail)."""
                isl = bass.ts(ib, IB)
                o_sb = o_sb_of.pop(ib)
                qTb_of.pop(ib, None)
                ob = obpool.tile([128, NJT, IB], bf16, name="ob")
                step = NJT // chunks
                for e0 in range(0, NJT, step):
                    for et in range(e0, e0 + step):
                        po = big.tile([128, IB], f32, tag="big", name="po")
                        for h in range(HL):
                            nc.tensor.matmul(
                                po[:],
                                lhsT=wo_sb[:, h, et * 128 : (et + 1) * 128],
                                rhs=o_sb[:, h, :],
                                start=(h == 0),
                                stop=(h == HL - 1),
                            )
                        if et % 2 == 0:
                            nc.scalar.activation(ob[:, et, :], po[:], Copy)
                        else:
                            nc.vector.tensor_copy(ob[:, et, :], po[:])
                        yield
                    nc.sync.dma_start(
                        out=outT_d.ap()[
                            e0 * 128 : (e0 + step) * 128, isl
                        ].rearrange("(e p) i -> p e i", p=128),
                        in_=ob[:, e0 : e0 + step, :],
                    )

            # Software-pipelined schedule: the next block's pass-A projection
            # units are interleaved INTO attention(ib) as PE filler, its
            # v-projection (pass B) fills the softmax-tail -> output join,
            # and x prefetches run a full step ahead. The FINAL block has no
            # next projections, so the previous block's phase 3 is deferred
            # and interleaved there instead (the last phase 2 is otherwise
            # vector-bound: exp+mul+sum-chain ~660ns/tile vs PE ~426ns/tile).
            blocks = [ib for _rep in range(repeat) for ib in range(NIB)]
            emit_xt_dma(blocks[0], first=True)
            for _ in phase1_passA_units(blocks[0]):
                pass
            if len(blocks) > 1:
                emit_xt_dma(blocks[1])
            emit_phase1_passB(blocks[0])
            for n, ib in enumerate(blocks):
                nxt = blocks[n + 1] if n + 1 < len(blocks) else None
                if nxt is not None:
                    emit_phase2(ib, filler=phase1_passA_units(nxt), ftotal=NJT + 1)
                    emit_phase1_passB(nxt)
                    if n + 2 < len(blocks):
                        emit_xt_dma(blocks[n + 2])
                    if n + 2 == len(blocks):
                        continue  # defer phase3(ib) into the final phase2
                    for _ in phase3_units(ib):
                        pass
                else:
                    prev = blocks[n - 1] if n > 0 else None
                    fil = phase3_units(prev) if prev is not None else None
                    emit_phase2(ib, filler=fil, ftotal=NJT)
                    for _ in phase3_units(ib, chunks=4):
                        pass

    nc.compile()
    _PROGRAM_CACHE[key] = nc
    return nc


# ------------------------------------------------------------------- kernel
def prepare(x, mask, wq, wk, wv, wo, shape, scale, loc, start_pos):
    """Host prep: build/cache program and per-core input maps."""
    x32 = np.asarray(x, np.float32).reshape(S, DIM)
    m2 = np.asarray(mask, np.float32).reshape(S, S)
    wq32 = np.asarray(wq, np.float32)
    wk32 = np.asarray(wk, np.float32)
    wv32 = np.asarray(wv, np.float32)
    wo32 = np.asarray(wo, np.float32)
    shape = np.asarray(shape, np.float32)
    scale = np.asarray(scale, np.float32)
    loc = np.asarray(loc, np.float32)
    sp = int(start_pos)

    phi = _toeplitz_profile(m2)
    fold_mask = phi is not None
    kept = _kept_tiles(m2)
    kept_key = _banded_kept(kept, shape, scale, loc)

    LAST_RUN_INFO["build_args"] = (kept_key, fold_mask)
    nc = _build_program(kept_key, fold_mask)

    bf = ml_dtypes.bfloat16
    xT = np.ascontiguousarray(x32.T).astype(bf)
    inv_s = np.float32(1.0 / math.sqrt(HD))
    maskT = None if fold_mask else np.ascontiguousarray(m2.T)

    in_maps = []
    for c in range(N_CORES):
        heads = HEADS_OF_CORE[c]
        cols = np.concatenate(
            [np.arange(h * HD, (h + 1) * HD) for h in heads]
        )
        im = {
            "xT": xT,
            "wq": np.ascontiguousarray(wq32[:, cols] * inv_s).astype(bf),
            "wk": np.ascontiguousarray(wk32[:, cols]).astype(bf),
            "wv": np.ascontiguousarray(wv32[:, cols]).astype(bf),
            "wo": np.ascontiguousarray(wo32[cols, :]).astype(bf),
            "g": np.stack(
                [_eg_table(h, shape, scale, loc, sp, phi) for h in heads]
            ),
        }
        if not fold_mask:
            im["maskT"] = maskT
        in_maps.append(im)
    return nc, in_maps


def _reduce(results):
    acc = results[0]["outT"].astype(np.float32)
    for c in range(1, N_CORES):
        acc = acc + results[c]["outT"].astype(np.float32)
    return np.ascontiguousarray(acc.T)[None].astype(np.float32)


_RUNNER_CACHE = {}


def _get_runner(nc):
    """Build (once) a reusable jitted 8-core runner for the program `nc`.
    Mirrors bass2jax.run_bass_via_pjrt's multi-core path without output
    donation (outT is fully written by the kernel) so it can be re-invoked."""
    if id(nc) in _RUNNER_CACHE:
        return _RUNNER_CACHE[id(nc)]

    import jax
    from jax.sharding import Mesh, NamedSharding, PartitionSpec

    from jax.experimental.shard_map import shard_map
    from concourse import mybir
    from concourse.bass2jax import (
        _bass_exec_p,
        install_neuronx_cc_hook,
        partition_id_tensor,
    )

    install_neuronx_cc_hook()
    partition_name = nc.partition_id_tensor.name if nc.partition_id_tensor else None

    in_names, out_names, out_avals = [], [], []
    for alloc in nc.m.functions[0].allocations:
        if not isinstance(alloc, mybir.MemoryLocationSet):
            continue
        name = alloc.memorylocations[0].name
        if alloc.kind == "ExternalInput":
            if name != partition_name:
                in_names.append(name)
        elif alloc.kind == "ExternalOutput":
            out_names.append(name)
            out_avals.append(
                jax.core.ShapedArray(
                    tuple(alloc.tensor_shape), mybir.dt.np(alloc.dtype)
                )
            )
    n_params = len(in_names)
    all_names = in_names + out_names
    if partition_name is not None:
        all_names = all_names + [partition_name]

    def _body(*args):
        operands = list(args)
        if partition_name is not None:
            operands.append(partition_id_tensor())
        return tuple(
            _bass_exec_p.bind(
                *operands,
                out_avals=tuple(out_avals),
                in_names=tuple(all_names),
                out_names=tuple(out_names),
                lowering_input_output_aliases=(),
                sim_require_finite=True,
                sim_require_nnan=True,
                nc=nc,
            )
        )

    devices = jax.devices()[:N_CORES]
    mesh = Mesh(np.asarray(devices), ("core",))
    sharded = jax.jit(
        shard_map(
            _body,
            mesh=mesh,
            in_specs=(PartitionSpec("core"),) * (n_params + len(out_names)),
            out_specs=(PartitionSpec("core"),) * len(out_names),
            check_rep=False,
        ),
        keep_unused=True,
    )
    sh = NamedSharding(mesh, PartitionSpec("core"))

    def run(in_maps):
        concat_in = [
            np.concatenate(
                [np.asarray(in_maps[c][nm]) for c in range(N_CORES)], axis=0
            )
            for nm in in_names
        ]
        concat_zero = [
            np.zeros((N_CORES * av.shape[0], *av.shape[1:]), av.dtype)
            for av in out_avals
        ]
        dev_args = [jax.device_put(a, sh) for a in concat_in + concat_zero]
        out_arrs = sharded(*dev_args)
        return [
            {
                nm: np.asarray(out_arrs[i]).reshape(
                    N_CORES, *out_avals[i].shape
                )[c]
                for i, nm in enumerate(out_names)
            }
            for c in range(N_CORES)
        ]

    _RUNNER_CACHE[id(nc)] = run
    return run


def kernel(x, mask, wq, wk, wv, wo, shape, scale, loc, start_pos):
    nc, in_maps = prepare(x, mask, wq, wk, wv, wo, shape, scale, loc, start_pos)

    if os.environ.get("KBA_SIM", "0") == "1":
        from concourse import bass_interp

        n_sim = int(os.environ.get("KBA_SIM_CORES", str(N_CORES)))
        sim = bass_interp.MultiCoreSim(nc, n_sim)
        for c in range(n_sim):
            for k, v in in_maps[c].items():
                sim.cores[c].tensor(k)[:] = v
        sim.simulate()
        results = [
            {"outT": np.array(sim.cores[c].tensor("outT"))}
            for c in range(n_sim)
        ] + [
            {"outT": np.zeros((S, S), ml_dtypes.bfloat16)}
            for _ in range(N_CORES - n_sim)
        ]
        LAST_RUN_INFO["exec_time_ns"] = None
    else:
        results = _get_runner(nc)(in_maps)
        LAST_RUN_INFO["exec_time_ns"] = None

    LAST_RUN_INFO["results"] = results
    return _reduce(results)
